# revision 1
# baseline (speedup 1.0000x reference)
"""Trainium2 Bass kernel: causal multi-head self-attention block (pre-LN).

Full module computed on 8 NeuronCores:
    xn = LayerNorm(x); q = xn@Wq.T+bq; k,v = xn@Wkv.T+bkv
    out = softmax(mask(q k^T / sqrt(dh))) v @ Wo.T + bo + x

Sharding: core = batch_index * 2 + head_half.  Each core handles one batch
element and 8 of the 16 heads (column-parallel QKV, row-parallel Wo), emits a
partial [S, D] output including half the residual; host sums core pairs and
adds bo.

v3: fp8e4 DoubleRow matmuls for QKV/AV/output projections (halves PE
instruction count per MAC).  QK stays bf16 (K=64; fp8 gives no gain there).
Weights are scaled x16 before the fp8 cast to clear the e4m3 subnormal range;
the x1/16 folds into downstream scalars; the host passes x/2 so the residual
add is a single fused op (LN epsilon quartered to compensate exactly).  The
next chunk's LayerNorm/transpose/projection work is queued as closures and
popped between attention steps so the PE always has independent work while
the Scalar engine streams the exp() chain.
"""

import os
import sys

import numpy as np

sys.path.insert(0, "/opt/trn_rl_repo")

B, S, D, H = 4, 2048, 1024, 16
DH = D // H            # 64
HL = H // 2            # heads per core: 8
OH = HL * DH           # per-core head features: 512
EPS = 1e-5
P = 128                # SBUF partitions
NST = S // P           # 16 s-tiles
NFT = D // P           # 8 feature tiles
NFP = NFT // 2         # 4 paired feature tiles (DoubleRow)
NOT = OH // P          # 4 o-tiles (per-core head features)
NOP = NOT // 2         # 2 paired o-tiles
NVP = NST // 2         # 8 paired v key-tiles
QS = 512               # query super-tile (matmul moving free dim)
NQS = S // QS          # 4
KT_PER_QS = QS // P    # 4 k-tiles per q-super
VW = 80                # per-head stride in the padded V tiles (16B-aligned)
ESC = 0.125 / 256.0    # exp scale: 1/sqrt(DH) plus 1/(16*16) weight descale

_CACHE = {}


def _build_nc():
    import concourse.bass as bass
    import concourse.bacc as bacc
    import concourse.tile as tile
    from concourse import mybir

    f32 = mybir.dt.float32
    bf16 = mybir.dt.bfloat16
    fp8 = mybir.dt.float8e4
    Alu = mybir.AluOpType
    Act = mybir.ActivationFunctionType
    DR = mybir.MatmulPerfMode.DoubleRow

    nc = bacc.Bacc("TRN2", target_bir_lowering=False, debug=False, num_devices=8)

    # ---- DRAM I/O (per-core shard shapes; weights pre-paired + fp8, x16) ----
    x_d = nc.dram_tensor("x", [S, D], f32, kind="ExternalInput").ap()  # x/2
    wq_d = nc.dram_tensor("wqt", [NFP * P, 2, OH], fp8, kind="ExternalInput").ap()
    wk_d = nc.dram_tensor("wkt", [NFP * P, 2, OH], fp8, kind="ExternalInput").ap()
    wv_d = nc.dram_tensor("wvt", [NFP * P, 2, OH], fp8, kind="ExternalInput").ap()
    wo_d = nc.dram_tensor("wot", [NOP * P, 2, D], fp8, kind="ExternalInput").ap()
    bq_d = nc.dram_tensor("bq", [P, NOT], f32, kind="ExternalInput").ap()   # x16
    bk_d = nc.dram_tensor("bk", [P, NOT], f32, kind="ExternalInput").ap()   # x16
    bv_d = nc.dram_tensor("bv", [OH], f32, kind="ExternalInput").ap()
    pad_d = nc.dram_tensor("pad01", [P, NST], f32, kind="ExternalInput").ap()
    out_d = nc.dram_tensor("out", [S, D], f32, kind="ExternalOutput").ap()
    debug = bool(os.environ.get("KERNEL_DEBUG"))
    if debug:
        dbg = {}
        for n, shp, dt in (("xnT0", [P, 2, S], fp8), ("qT0", [P, S], bf16),
                           ("kT0", [P, S], bf16), ("vp0", [P, 2, HL * VW], fp8),
                           ("oT0", [P, 2, S], fp8)):
            dbg[n] = nc.dram_tensor(f"dbg_{n}", shp, dt,
                                    kind="ExternalOutput").ap()

    def bcast(ap_1d, n):
        # [n] dram vector -> [P, n] partition-broadcast DMA source
        return bass.AP(tensor=ap_1d.tensor, offset=ap_1d.offset,
                       ap=[[0, P], [1, n]])

    def headview(ap_2d, stride, width):
        # [P, *] AP -> [P, HL, width] view with per-head stride
        return bass.AP(tensor=ap_2d.tensor, offset=ap_2d.offset,
                       ap=[ap_2d.ap[0], [stride, HL], [1, width]])

    with tile.TileContext(nc) as tc:
        with (
            tc.tile_pool(name="res", bufs=1) as res,       # resident tensors
            tc.tile_pool(name="small", bufs=4) as small,
        ):
            # ---------- constants ----------
            vb_sb = res.tile([P, OH], f32, tag="vb_sb")
            pad_sb = res.tile([P, NST], f32, tag="pad_sb")
            zero_sb = res.tile([P, 1], f32, tag="zero_sb")
            nc.vector.memset(zero_sb, 0.0)
            ident_b = res.tile([P, P], bf16, tag="ident_b")
            nc.gpsimd.memset(ident_b, 0.0)
            nc.gpsimd.affine_select(
                out=ident_b, in_=ident_b, compare_op=Alu.not_equal, fill=1.0,
                base=0, pattern=[[-1, P]], channel_multiplier=1)
            bq_sb = res.tile([P, NOT], f32, tag="bq_sb")
            bk_sb = res.tile([P, NOT], f32, tag="bk_sb")
            eps_sb = res.tile([P, 1], f32, tag="eps_sb")
            nc.vector.memset(eps_sb, EPS / 4.0)   # x/2 halves sigma; exact

            # ---------- resident big tensors ----------
            xnT = [res.tile([P, 2, S], fp8, tag=f"xnT{j}", name=f"xnT{j}")
                   for j in range(NFP)]
            qT = [res.tile([P, S], bf16, tag=f"qT{t}", name=f"qT{t}")
                  for t in range(NOT)]
            kT = [res.tile([P, S], bf16, tag=f"kT{t}", name=f"kT{t}")
                  for t in range(NOT)]
            # V pairs augmented with a ones column per head at h*VW+64;
            # VW=80 keeps the DoubleRow ldweights 16B-aligned
            vp = [res.tile([P, 2, HL * VW], fp8, tag=f"vp{i}",
                           name=f"vp{i}") for i in range(NVP)]
            oT = [res.tile([P, 2, S], fp8, tag=f"oT{t}", name=f"oT{t}")
                  for t in range(NOP)]
            wqT = [res.tile([P, 2, OH], fp8, tag=f"wqT{j}", name=f"wqT{j}")
                   for j in range(NFP)]
            wkT = [res.tile([P, 2, OH], fp8, tag=f"wkT{j}", name=f"wkT{j}")
                   for j in range(NFP)]
            wvT = [res.tile([P, 2, OH], fp8, tag=f"wvT{j}", name=f"wvT{j}")
                   for j in range(NFP)]
            woT = [res.tile([P, 2, D], fp8, tag=f"woT{t}", name=f"woT{t}")
                   for t in range(NOP)]

            with (
                tc.tile_pool(name="pj_psum", bufs=2, space="PSUM") as pp,
                tc.tile_pool(name="s_psum", bufs=2, space="PSUM") as sp,
                tc.tile_pool(name="o_psum", bufs=2, space="PSUM") as op,
                tc.tile_pool(name="pt", bufs=6) as ptp,
                tc.tile_pool(name="nrm", bufs=2) as nrm,
                tc.tile_pool(name="ld", bufs=4) as ld,
                tc.tile_pool(name="tmp", bufs=3) as tmp,
                tc.tile_pool(name="lde", bufs=8) as lde,
                tc.tile_pool(name="tmpe", bufs=3) as tmpe,
            ):
                for i in range(NVP):
                    nc.gpsimd.memset(vp[i], 1.0)

                xns = {}

                def ln_compute(st):
                    x_t = ld.tile([P, D], f32, tag="x_ln")
                    nc.sync.dma_start(out=x_t,
                                      in_=x_d[st * P:(st + 1) * P, :])
                    stats = small.tile([P, 2, 6], f32, tag="stats")
                    for sg in range(2):
                        nc.vector.bn_stats(
                            out=stats[:, sg, :],
                            in_=x_t[:, sg * 512:(sg + 1) * 512])
                    mv = small.tile([P, 2], f32, tag="mv")
                    nc.vector.bn_aggr(out=mv, in_=stats)
                    rstd = small.tile([P, 1], f32, tag="rstd")
                    nc.scalar.activation(out=rstd, in_=mv[:, 1:2],
                                         func=Act.Sqrt, bias=eps_sb,
                                         scale=1.0)
                    nc.vector.reciprocal(out=rstd, in_=rstd)
                    mb = small.tile([P, 1], f32, tag="mb")
                    nc.vector.tensor_scalar(
                        out=mb, in0=mv[:, 0:1], scalar1=rstd, scalar2=-1.0,
                        op0=Alu.mult, op1=Alu.mult)
                    xn = tmp.tile([P, D], bf16, tag="xn", bufs=9)
                    nc.vector.tensor_scalar(
                        out=xn, in0=x_t, scalar1=rstd, scalar2=mb,
                        op0=Alu.mult, op1=Alu.add)
                    xns[st] = xn

                def ln_transpose(st):
                    xn = xns[st]
                    for j in range(NFT):
                        ps = pp.tile([P, P], bf16, tag="pj")
                        nc.tensor.transpose(
                            ps, xn[:, j * P:(j + 1) * P], ident_b)
                        dst = xnT[j // 2][:, j % 2, st * P:(st + 1) * P]
                        if j % 2 == 0:
                            nc.vector.tensor_copy(out=dst, in_=ps)
                        else:
                            nc.scalar.copy(out=dst, in_=ps)

                def qk_proj(c, wT, dst, bias, t):
                    ps = pp.tile([P, QS], f32, tag="pj")
                    for j in range(NFP):
                        nc.tensor.matmul(
                            ps,
                            lhsT=wT[j][:, :, t * P:(t + 1) * P],
                            rhs=xnT[j][:, :, c * QS:(c + 1) * QS],
                            start=(j == 0), stop=(j == NFP - 1),
                            perf_mode=DR)
                    nc.vector.tensor_scalar_add(
                        out=dst[t][:, c * QS:(c + 1) * QS],
                        in0=ps, scalar1=bias[:, t:t + 1])

                def v_proj(st):
                    ps = pp.tile([P, OH], f32, tag="pj")
                    for j in range(NFP):
                        nc.tensor.matmul(
                            ps,
                            lhsT=xnT[j][:, :, st * P:(st + 1) * P],
                            rhs=wvT[j],
                            start=(j == 0), stop=(j == NFP - 1),
                            perf_mode=DR)
                    vsl = vp[st // 2][:, st % 2, :]
                    nc.vector.scalar_tensor_tensor(
                        out=headview(vsl, VW, DH),
                        in0=headview(ps[:, :], DH, DH),
                        scalar=0.0625,
                        in1=headview(vb_sb[:, :], DH, DH),
                        op0=Alu.mult, op1=Alu.add)
                    nc.vector.tensor_scalar_mul(
                        out=vsl, in0=vsl, scalar1=pad_sb[:, st:st + 1])

                def outproj_prefetch(st, mc):
                    x_sk = lde.tile([P, QS], f32, tag="x_sk")
                    nc.sync.dma_start(
                        out=x_sk,
                        in_=x_d[st * P:(st + 1) * P, mc * QS:(mc + 1) * QS])
                    return x_sk

                def outproj_compute(st, mc, x_sk):
                    ps = pp.tile([P, QS], f32, tag="pj")
                    for ot in range(NOP):
                        nc.tensor.matmul(
                            ps,
                            lhsT=oT[ot][:, :, st * P:(st + 1) * P],
                            rhs=woT[ot][:, :, mc * QS:(mc + 1) * QS],
                            start=(ot == 0), stop=(ot == NOP - 1),
                            perf_mode=DR)
                    y_sb = tmpe.tile([P, QS], f32, tag="y_sb")
                    nc.vector.scalar_tensor_tensor(
                        out=y_sb, in0=ps, scalar=0.0625, in1=x_sk,
                        op0=Alu.mult, op1=Alu.add)
                    nc.sync.dma_start(
                        out=out_d[st * P:(st + 1) * P,
                                  mc * QS:(mc + 1) * QS],
                        in_=y_sb)

                def chunk_helpers(c):
                    # PE-feeding closures for chunk c's transposes+projections
                    work = []
                    for st in range(c * KT_PER_QS, (c + 1) * KT_PER_QS):
                        work.append(lambda st=st: ln_transpose(st))
                        work.append(lambda st=st: v_proj(st))
                    for (wT, dst, bias) in ((wqT, qT, bq_sb),
                                            (wkT, kT, bk_sb)):
                        for t in range(NOT):
                            work.append(
                                lambda wT=wT, dst=dst, bias=bias, t=t, c=c:
                                qk_proj(c, wT, dst, bias, t))
                    return work

                # ---------- prologue (v2 order) ----------
                for st in range(2 * KT_PER_QS):
                    ln_compute(st)
                nc.sync.dma_start(out=vb_sb, in_=bcast(bv_d, OH))
                nc.sync.dma_start(out=pad_sb, in_=pad_d)
                nc.sync.dma_start(out=bq_sb, in_=bq_d)
                nc.sync.dma_start(out=bk_sb, in_=bk_d)
                for j in range(NFP):
                    nc.sync.dma_start(out=wqT[j], in_=wq_d[j * P:(j + 1) * P, :, :])
                    nc.sync.dma_start(out=wkT[j], in_=wk_d[j * P:(j + 1) * P, :, :])
                    nc.sync.dma_start(out=wvT[j], in_=wv_d[j * P:(j + 1) * P, :, :])
                for t in range(NOP):
                    nc.sync.dma_start(out=woT[t], in_=wo_d[t * P:(t + 1) * P, :, :])

                outproj_q = []
                for w in chunk_helpers(0):
                    w()
                helper_q = chunk_helpers(1)
                for c in range(NQS):
                    # LN for chunk c+2 races on DVE/GpSimd during attention
                    if c + 2 < NQS:
                        for st in range((c + 2) * KT_PER_QS,
                                        (c + 3) * KT_PER_QS):
                            ln_compute(st)

                    qs = c
                    nkt = (qs + 1) * KT_PER_QS

                    def qk(h, kt, s_ps):
                        hq = qT[h // 2][(h % 2) * DH:(h % 2) * DH + DH, :]
                        hk = kT[h // 2][(h % 2) * DH:(h % 2) * DH + DH, :]
                        nc.tensor.matmul(
                            s_ps,
                            lhsT=hk[:, kt * P:(kt + 1) * P],
                            rhs=hq[:, qs * QS:(qs + 1) * QS],
                            start=True, stop=True, skip_group_check=True)

                    def expmask(kt0, s_ps):
                        pt = ptp.tile([P, 2, QS], fp8, tag="pt", bufs=4)
                        for i in range(2):
                            nc.scalar.activation(
                                out=pt[:, i, :], in_=s_ps[:, i, :],
                                func=Act.Exp, bias=zero_sb, scale=ESC)
                        for i in range(2):
                            kt = kt0 + i
                            if kt >= qs * KT_PER_QS:  # diagonal region
                                nc.gpsimd.affine_select(
                                    out=pt[:, i, :], in_=pt[:, i, :],
                                    compare_op=Alu.is_ge, fill=0.0,
                                    base=qs * QS - kt * P,
                                    pattern=[[1, QS]], channel_multiplier=-1)
                        return pt

                    def av(h, kt0, pt, o_ps):
                        nc.tensor.matmul(
                            o_ps,
                            lhsT=vp[kt0 // 2][:, :,
                                              h * VW:h * VW + DH + 2],
                            rhs=pt,
                            start=(kt0 == 0), stop=(kt0 + 2 == nkt),
                            perf_mode=DR, skip_group_check=True)

                    def normalize(h, o_ps):
                        den_sb = nrm.tile([1, QS], f32, tag="den_sb")
                        nc.vector.tensor_copy(den_sb, o_ps[DH:DH + 1, :])
                        dbc = nrm.tile([DH, QS], f32, tag="dbc")
                        nc.vector.reciprocal_approx_fast(
                            out=dbc[0:1, :], in_=den_sb)
                        nc.gpsimd.partition_broadcast(dbc, dbc[0:1, :])
                        nc.vector.tensor_mul(
                            out=oT[h // 4][(h % 2) * DH:(h % 2) * DH + DH,
                                           (h // 2) % 2,
                                           qs * QS:(qs + 1) * QS],
                            in0=o_ps[0:DH, :], in1=dbc)

                    # spread queued helper + outproj work across the
                    # attention iterations so the PE never starves
                    total_slots = (qs + 1) * 2 * (HL // 2)
                    total_work = len(helper_q) + len(outproj_q)
                    total_popped = [0]

                    def pop_some(done_slots):
                        want = (total_work * done_slots) // total_slots
                        popped = total_popped[0]
                        while popped < want:
                            if helper_q:
                                helper_q.pop(0)()
                            elif outproj_q:
                                outproj_q.pop(0)()
                            else:
                                break
                            popped += 1
                        total_popped[0] = popped

                    slot = 0
                    for h0 in range(0, HL, 2):
                        h1 = h0 + 1
                        o_ps0 = op.tile([DH + 2, QS], f32, tag="o_ps")
                        o_ps1 = op.tile([DH + 2, QS], f32, tag="o_ps")
                        for kt0 in range(0, nkt, 2):
                            s0 = sp.tile([P, 2, QS], f32, tag="s_ps")
                            qk(h0, kt0, s0[:, 0, :])
                            qk(h0, kt0 + 1, s0[:, 1, :])
                            pt0 = expmask(kt0, s0)
                            s1 = sp.tile([P, 2, QS], f32, tag="s_ps")
                            qk(h1, kt0, s1[:, 0, :])
                            qk(h1, kt0 + 1, s1[:, 1, :])
                            pt1 = expmask(kt0, s1)
                            av(h0, kt0, pt0, o_ps0)
                            av(h1, kt0, pt1, o_ps1)
                            slot += 1
                            pop_some(slot)
                        normalize(h0, o_ps0)
                        normalize(h1, o_ps1)
                    while helper_q:
                        helper_q.pop(0)()
                    while outproj_q:
                        outproj_q.pop(0)()

                    if c + 2 < NQS:
                        helper_q = chunk_helpers(c + 2)
                    for st in range(qs * KT_PER_QS, (qs + 1) * KT_PER_QS):
                        for mc in range(2):
                            x_sk = outproj_prefetch(st, mc)
                            outproj_q.append(
                                lambda st=st, mc=mc, x_sk=x_sk:
                                outproj_compute(st, mc, x_sk))

                # drain remaining output-projection work (last q-super)
                for w in outproj_q:
                    w()

                if debug:
                    nc.sync.dma_start(out=dbg["xnT0"], in_=xnT[0])
                    nc.sync.dma_start(out=dbg["qT0"], in_=qT[0])
                    nc.sync.dma_start(out=dbg["kT0"], in_=kT[0])
                    nc.sync.dma_start(out=dbg["vp0"], in_=vp[0])
                    nc.sync.dma_start(out=dbg["oT0"], in_=oT[0])

    nc.compile()
    return nc


def _get_nc():
    if "nc" not in _CACHE:
        _CACHE["nc"] = _build_nc()
    return _CACHE["nc"]


def make_in_maps(x, key_val_lengths, Wq, bq, Wkv, bkv, Wo, bo, ln_g, ln_b):
    import ml_dtypes
    fp8 = ml_dtypes.float8_e4m3

    x = np.ascontiguousarray(np.asarray(x, dtype=np.float32))
    lens = np.asarray(key_val_lengths).astype(np.int64)
    Wq = np.asarray(Wq, dtype=np.float32)
    Wkv = np.asarray(Wkv, dtype=np.float32)
    Wo = np.asarray(Wo, dtype=np.float32)
    bq = np.asarray(bq, dtype=np.float32)
    bkv = np.asarray(bkv, dtype=np.float32)
    ln_g = np.asarray(ln_g, dtype=np.float32)
    ln_b = np.asarray(ln_b, dtype=np.float32)

    # fold LayerNorm gain into the projection weights and shift into the
    # biases (exact algebra): q = ((x-mu)rstd*g + b) @ Wq.T + bq
    #                           = xhat @ (Wq*g).T + (Wq@b + bq)
    g64 = ln_g.astype(np.float64)
    b64 = ln_b.astype(np.float64)
    Wq64 = Wq.astype(np.float64) * g64
    Wk64 = Wkv[:D].astype(np.float64) * g64
    Wv64 = Wkv[D:].astype(np.float64) * g64
    bq_f = (Wq.astype(np.float64) @ b64 + bq).astype(np.float32)
    bk_f = (Wkv[:D].astype(np.float64) @ b64 + bkv[:D]).astype(np.float32)
    bv_f = (Wkv[D:].astype(np.float64) @ b64 + bkv[D:]).astype(np.float32)

    def pair_rows(wT, width):
        # [D or OH, width] fp8 -> [rows/2, 2, width] with 128-row pairs
        # interleaved for the DoubleRow contraction layout
        n = wT.shape[0]
        return np.ascontiguousarray(
            wT.reshape(n // 256, 2, P, width).transpose(0, 2, 1, 3)
            .reshape(n // 2, 2, width))

    in_maps = []
    for core in range(8):
        b, half = divmod(core, 2)
        sl = slice(half * OH, (half + 1) * OH)
        pad01 = (np.arange(S) < lens[b]).astype(np.float32)
        in_maps.append({
            "x": x[b] * 0.5,
            "wqt": pair_rows((Wq64[sl].T * 16.0).astype(fp8), OH),
            "wkt": pair_rows((Wk64[sl].T * 16.0).astype(fp8), OH),
            "wvt": pair_rows((Wv64[sl].T * 16.0).astype(fp8), OH),
            "wot": pair_rows((Wo[:, sl].T * 16.0).astype(fp8), D),
            "bq": np.ascontiguousarray((16.0 * bq_f[sl]).reshape(NOT, P).T),
            "bk": np.ascontiguousarray((16.0 * bk_f[sl]).reshape(NOT, P).T),
            "bv": np.ascontiguousarray(bv_f[sl]),
            "pad01": np.ascontiguousarray(pad01.reshape(NST, P).T),
        })
    return in_maps


def kernel(**inputs):
    from concourse.bass_utils import run_bass_kernel_spmd

    trace = bool(os.environ.get("KERNEL_TRACE"))
    if trace:
        try:
            import antenv.axon_hooks  # noqa: F401  (profiling shim present?)
        except ImportError:
            trace = False
    nc = _get_nc()
    in_maps = make_in_maps(**inputs)
    res = run_bass_kernel_spmd(
        nc, in_maps, core_ids=list(range(8)), trace=trace)
    _CACHE["last_results"] = res
    bo = np.asarray(inputs["bo"], dtype=np.float32)
    y = np.empty((B, S, D), dtype=np.float32)
    for b in range(B):
        y[b] = res.results[2 * b]["out"] + res.results[2 * b + 1]["out"] + bo
    return y



# revision 8
# speedup vs baseline: 1.0798x; 1.0798x over previous
"""Trainium2 Bass kernel: causal multi-head self-attention block (pre-LN).

Full module computed on 8 NeuronCores:
    xn = LayerNorm(x); q = xn@Wq.T+bq; k,v = xn@Wkv.T+bkv
    out = softmax(mask(q k^T / sqrt(dh))) v @ Wo.T + bo + x
Sharding: core = batch_index * 2 + head_half.  Each core handles one batch
element and 8 of the 16 heads (column-parallel QKV, row-parallel Wo), emits a
partial [S, D] output including half the residual; host sums core pairs and
adds bo (with the V-bias contribution folded in on host: exact algebra).

v4: single ACT table set (rstd = exp(-0.5 ln(var+eps))); one wide ACTIVATE per
[P,2,QS] score tile, windowed past the fully-masked half of diagonal pairs;
causal mask as a static-fp8-tile multiply (r2 pairs draw from a zero-
initialized pt pool so the skipped window stays 0); x resident in SBUF as
bf16; software-pipelined slots (QK of slot i+1 -> paced helper work -> AV of
slot i) so the PE always has independent queued work while Scalar streams exp.
"""

import os
import sys

import numpy as np

sys.path.insert(0, "/opt/trn_rl_repo")

B, S, D, H = 4, 2048, 1024, 16
DH = D // H            # 64
HL = H // 2            # heads per core: 8
OH = HL * DH           # per-core head features: 512
EPS = 1e-5
P = 128                # SBUF partitions
NST = S // P           # 16 s-tiles
NFT = D // P           # 8 feature tiles
NFP = NFT // 2         # 4 paired feature tiles (DoubleRow)
NOT = OH // P          # 4 o-tiles (per-core head features)
NOP = NOT // 2         # 2 paired o-tiles
NVP = NST // 2         # 8 paired v key-tiles
QS = 512               # query super-tile (matmul moving free dim)
NQS = S // QS          # 4
KT_PER_QS = QS // P    # 4 k-tiles per q-super
VW = 80                # per-head stride in the padded V tiles (16B-aligned)
ESC = 0.125 / 256.0    # exp scale: 1/sqrt(DH) plus 1/(16*16) weight descale

_CACHE = {}


def _build_nc():
    import concourse.bass as bass
    import concourse.bacc as bacc
    import concourse.tile as tile
    from concourse import mybir

    f32 = mybir.dt.float32
    bf16 = mybir.dt.bfloat16
    fp8 = mybir.dt.float8e4
    Alu = mybir.AluOpType
    Act = mybir.ActivationFunctionType
    DR = mybir.MatmulPerfMode.DoubleRow

    nc = bacc.Bacc("TRN2", target_bir_lowering=False, debug=False, num_devices=8)

    # ---- DRAM I/O (per-core shard shapes; weights pre-paired + fp8, x16) ----
    x_d = nc.dram_tensor("x", [S, D], bf16, kind="ExternalInput").ap()  # x/2
    wq_d = nc.dram_tensor("wqt", [NFP * P, 2, OH], fp8, kind="ExternalInput").ap()
    wk_d = nc.dram_tensor("wkt", [NFP * P, 2, OH], fp8, kind="ExternalInput").ap()
    wv_d = nc.dram_tensor("wvt", [NFP * P, 2, OH], fp8, kind="ExternalInput").ap()
    wo_d = nc.dram_tensor("wot", [NOP * P, 2, D], fp8, kind="ExternalInput").ap()
    bq_d = nc.dram_tensor("bq", [P, NOT], f32, kind="ExternalInput").ap()   # x16
    bk_d = nc.dram_tensor("bk", [P, NOT], f32, kind="ExternalInput").ap()   # x16
    pad_d = nc.dram_tensor("pad01", [P, NST], f32, kind="ExternalInput").ap()
    out_d = nc.dram_tensor("out", [S, D], f32, kind="ExternalOutput").ap()

    def headview(ap_2d, stride, width):
        # [P, *] AP -> [P, HL, width] view with per-head stride
        return bass.AP(tensor=ap_2d.tensor, offset=ap_2d.offset,
                       ap=[ap_2d.ap[0], [stride, HL], [1, width]])

    with tile.TileContext(nc) as tc:
        with (
            tc.tile_pool(name="res", bufs=1) as res,       # resident tensors
            tc.tile_pool(name="small", bufs=4) as small,
        ):
            # ---------- constants ----------
            pad_sb = res.tile([P, NST], f32, tag="pad_sb")
            zero_sb = res.tile([P, 1], f32, tag="zero_sb")
            nc.vector.memset(zero_sb, 0.0)
            ident_b = res.tile([P, P], bf16, tag="ident_b")
            nc.gpsimd.memset(ident_b, 0.0)
            nc.gpsimd.affine_select(
                out=ident_b, in_=ident_b, compare_op=Alu.not_equal, fill=1.0,
                base=0, pattern=[[-1, P]], channel_multiplier=1)
            # causal mask for a diagonal kt pair, relative to the pair's
            # query window: keep where col >= p + 128*i
            mask_sb = res.tile([P, 2, 2 * P], fp8, tag="mask_sb")
            nc.gpsimd.memset(mask_sb, 1.0)
            nc.gpsimd.affine_select(
                out=mask_sb, in_=mask_sb, compare_op=Alu.is_ge, fill=0.0,
                base=0, pattern=[[-P, 2], [1, 2 * P]], channel_multiplier=-1)
            bq_sb = res.tile([P, NOT], f32, tag="bq_sb")
            bk_sb = res.tile([P, NOT], f32, tag="bk_sb")
            eps_sb = res.tile([P, 1], f32, tag="eps_sb")
            nc.vector.memset(eps_sb, EPS / 4.0)   # x/2 quarters the variance

            # ---------- resident big tensors ----------
            x_res = [res.tile([P, D], bf16, tag=f"xr{st}", name=f"xr{st}")
                     for st in range(NST)]
            xnT = [res.tile([P, 2, S], fp8, tag=f"xnT{j}", name=f"xnT{j}")
                   for j in range(NFP)]
            qT = [res.tile([P, S], bf16, tag=f"qT{t}", name=f"qT{t}")
                  for t in range(NOT)]
            kT = [res.tile([P, S], bf16, tag=f"kT{t}", name=f"kT{t}")
                  for t in range(NOT)]
            # V pairs augmented with a pad column per head at h*VW+64;
            # VW=80 keeps the DoubleRow ldweights 16B-aligned
            vp = [res.tile([P, 2, HL * VW], fp8, tag=f"vp{i}",
                           name=f"vp{i}") for i in range(NVP)]
            oT = [res.tile([P, 2, S], fp8, tag=f"oT{t}", name=f"oT{t}")
                  for t in range(NOP)]
            wqT = [res.tile([P, 2, OH], fp8, tag=f"wqT{j}", name=f"wqT{j}")
                   for j in range(NFP)]
            wkT = [res.tile([P, 2, OH], fp8, tag=f"wkT{j}", name=f"wkT{j}")
                   for j in range(NFP)]
            wvT = [res.tile([P, 2, OH], fp8, tag=f"wvT{j}", name=f"wvT{j}")
                   for j in range(NFP)]
            woT = [res.tile([P, 2, D], fp8, tag=f"woT{t}", name=f"woT{t}")
                   for t in range(NOP)]

            with (
                tc.tile_pool(name="pj_psum", bufs=2, space="PSUM") as pp,
                tc.tile_pool(name="s_psum", bufs=2, space="PSUM") as sp,
                tc.tile_pool(name="o_psum", bufs=2, space="PSUM") as op,
                tc.tile_pool(name="pt", bufs=4) as ptp,
                tc.tile_pool(name="pt2", bufs=2) as pt2p,
                tc.tile_pool(name="nrm", bufs=2) as nrm,
                tc.tile_pool(name="tmp", bufs=3) as tmp,
                tc.tile_pool(name="ye", bufs=3) as yp,
            ):
                # ---------- prologue DMAs ----------
                nc.sync.dma_start(out=pad_sb, in_=pad_d)
                nc.sync.dma_start(out=bq_sb, in_=bq_d)
                nc.sync.dma_start(out=bk_sb, in_=bk_d)
                for st in range(8):
                    nc.sync.dma_start(out=x_res[st],
                                      in_=x_d[st * P:(st + 1) * P, :])
                for j in range(NFP):
                    nc.sync.dma_start(out=wqT[j], in_=wq_d[j * P:(j + 1) * P, :, :])
                    nc.sync.dma_start(out=wkT[j], in_=wk_d[j * P:(j + 1) * P, :, :])
                    nc.sync.dma_start(out=wvT[j], in_=wv_d[j * P:(j + 1) * P, :, :])
                for t in range(NOP):
                    nc.sync.dma_start(out=woT[t], in_=wo_d[t * P:(t + 1) * P, :, :])
                for st in range(8, NST):
                    nc.sync.dma_start(out=x_res[st],
                                      in_=x_d[st * P:(st + 1) * P, :])

                # V tiles: 1.0 everywhere, then the per-head pad column gets
                # the 0/1 pad value for its two key tiles (once, up front)
                for i in range(NVP):
                    nc.gpsimd.memset(vp[i], 1.0)
                for st in range(NST):
                    vsl = vp[st // 2][:, st % 2, :]
                    ones = bass.AP(tensor=vsl.tensor, offset=vsl.offset + DH,
                                   ap=[vsl.ap[0], [VW, HL], [1, 1]])
                    nc.gpsimd.tensor_scalar_mul(
                        out=ones, in0=ones, scalar1=pad_sb[:, st:st + 1])

                # pre-zero the diagonal-r2 pt buffers: their [0, 2P) window
                # is never written, so AV reads zeros there forever
                for _ in range(2):
                    z = pt2p.tile([P, 2, QS], fp8, tag="pt2")
                    nc.vector.memset(z, 0.0)

                xns = {}

                def ln_compute(st):
                    stats = small.tile([P, 2, 6], f32, tag="stats")
                    for sg in range(2):
                        nc.vector.bn_stats(
                            out=stats[:, sg, :],
                            in_=x_res[st][:, sg * 512:(sg + 1) * 512])
                    mv = small.tile([P, 2], f32, tag="mv")
                    nc.vector.bn_aggr(out=mv, in_=stats)
                    nc.vector.tensor_scalar_max(
                        out=mv[:, 1:2], in0=mv[:, 1:2], scalar1=0.0)
                    # rstd = exp(-0.5*ln(var+eps)): ln+exp live in one ACT
                    # table set, so no mid-kernel table swaps
                    lnv = small.tile([P, 1], f32, tag="lnv")
                    nc.scalar.activation(out=lnv, in_=mv[:, 1:2],
                                         func=Act.Ln, bias=eps_sb, scale=1.0)
                    rstd = small.tile([P, 1], f32, tag="rstd")
                    nc.scalar.activation(out=rstd, in_=lnv, func=Act.Exp,
                                         bias=zero_sb, scale=-0.5)
                    mb = small.tile([P, 1], f32, tag="mb")
                    nc.vector.tensor_scalar(
                        out=mb, in0=mv[:, 0:1], scalar1=rstd, scalar2=-1.0,
                        op0=Alu.mult, op1=Alu.mult)
                    xn = tmp.tile([P, D], bf16, tag="xn", bufs=9)
                    nc.vector.tensor_scalar(
                        out=xn, in0=x_res[st], scalar1=rstd, scalar2=mb,
                        op0=Alu.mult, op1=Alu.add)
                    xns[st] = xn

                def ln_transpose(st):
                    xn = xns.pop(st)
                    for j in range(NFT):
                        ps = pp.tile([P, P], bf16, tag="pj")
                        nc.tensor.transpose(
                            ps, xn[:, j * P:(j + 1) * P], ident_b)
                        nc.vector.tensor_copy(
                            out=xnT[j // 2][:, j % 2, st * P:(st + 1) * P],
                            in_=ps)

                def qk_proj(c, wT, dst, bias, t):
                    ps = pp.tile([P, QS], f32, tag="pj")
                    for j in range(NFP):
                        nc.tensor.matmul(
                            ps,
                            lhsT=wT[j][:, :, t * P:(t + 1) * P],
                            rhs=xnT[j][:, :, c * QS:(c + 1) * QS],
                            start=(j == 0), stop=(j == NFP - 1),
                            perf_mode=DR)
                    nc.vector.tensor_scalar_add(
                        out=dst[t][:, c * QS:(c + 1) * QS],
                        in0=ps, scalar1=bias[:, t:t + 1])

                def v_proj(st):
                    ps = pp.tile([P, OH], f32, tag="pj")
                    for j in range(NFP):
                        nc.tensor.matmul(
                            ps,
                            lhsT=xnT[j][:, :, st * P:(st + 1) * P],
                            rhs=wvT[j],
                            start=(j == 0), stop=(j == NFP - 1),
                            perf_mode=DR)
                    vsl = vp[st // 2][:, st % 2, :]
                    # v = raw*0.0625*pad (vb folded into bo on host)
                    nc.vector.tensor_scalar(
                        out=headview(vsl, VW, DH),
                        in0=headview(ps[:, :], DH, DH),
                        scalar1=0.0625, scalar2=pad_sb[:, st:st + 1],
                        op0=Alu.mult, op1=Alu.mult)

                def outproj(st, mc):
                    ps = pp.tile([P, QS], f32, tag="pj")
                    for ot in range(NOP):
                        nc.tensor.matmul(
                            ps,
                            lhsT=oT[ot][:, :, st * P:(st + 1) * P],
                            rhs=woT[ot][:, :, mc * QS:(mc + 1) * QS],
                            start=(ot == 0), stop=(ot == NOP - 1),
                            perf_mode=DR)
                    y_sb = yp.tile([P, QS], f32, tag="y_sb")
                    nc.vector.scalar_tensor_tensor(
                        out=y_sb, in0=ps, scalar=0.0625,
                        in1=x_res[st][:, mc * QS:(mc + 1) * QS],
                        op0=Alu.mult, op1=Alu.add)
                    nc.sync.dma_start(
                        out=out_d[st * P:(st + 1) * P,
                                  mc * QS:(mc + 1) * QS],
                        in_=y_sb)

                # ---------- helper work queue (PE filler) ----------
                # entries: (deadline_chunk, pe_cost_ns, fn)
                queue = []
                popped = [0.0]
                budget = [0.0]

                def push_prep(c):
                    c0 = c * KT_PER_QS
                    for st in range(c0, c0 + KT_PER_QS):
                        queue.append((c, 2200.0, lambda st=st: ln_transpose(st)))
                        queue.append((c, 964.0, lambda st=st: v_proj(st)))
                    for (wT, dst, bias) in ((wqT, qT, bq_sb),
                                            (wkT, kT, bk_sb)):
                        for t in range(NOT):
                            queue.append(
                                (c, 964.0,
                                 lambda wT=wT, dst=dst, bias=bias, t=t, c=c:
                                 qk_proj(c, wT, dst, bias, t)))

                def push_outproj(c):
                    for st in range(c * KT_PER_QS, (c + 1) * KT_PER_QS):
                        for mc in range(2):
                            queue.append(
                                (99, 600.0,
                                 lambda st=st, mc=mc: outproj(st, mc)))

                def pop_paced():
                    while queue and popped[0] < budget[0]:
                        _, cost, fn = queue.pop(0)
                        fn()
                        popped[0] += cost

                def drain_deadline(c):
                    keep = []
                    for ent in queue:
                        if ent[0] <= c:
                            ent[2]()
                            popped[0] += ent[1]
                        else:
                            keep.append(ent)
                    queue[:] = keep

                # ---------- attention slot machinery ----------
                def emit_qk(qs, hp, kt0):
                    # 4 QK matmuls; h0 rows 0-63 and h1 rows 64-127 run
                    # row-tile concurrent into separate PSUM tiles
                    tiles = []
                    for hh in range(2):
                        h = 2 * hp + hh
                        hq = qT[h // 2][(h % 2) * DH:(h % 2) * DH + DH, :]
                        hk = kT[h // 2][(h % 2) * DH:(h % 2) * DH + DH, :]
                        s_t = sp.tile([P, 2, QS], f32, tag="s_ps")
                        for i in range(2):
                            kt = kt0 + i
                            nc.tensor.matmul(
                                s_t[:, i, :],
                                lhsT=hk[:, kt * P:(kt + 1) * P],
                                rhs=hq[:, qs * QS:(qs + 1) * QS],
                                start=True, stop=True, skip_group_check=True)
                        tiles.append(s_t)
                    return tiles

                def emit_exp(qs, kt0, s_tiles):
                    diag = kt0 >= qs * KT_PER_QS
                    r2 = diag and (kt0 - qs * KT_PER_QS) == 2
                    ws = 2 * P if r2 else 0
                    pts = []
                    for s_t in s_tiles:
                        pool = pt2p if r2 else ptp
                        pt = pool.tile([P, 2, QS], fp8,
                                       tag="pt2" if r2 else "pt")
                        if r2:
                            nc.vector.memset(pt[:, :, :ws], 0.0)
                        nc.scalar.activation(
                            out=pt[:, :, ws:], in_=s_t[:, :, ws:],
                            func=Act.Exp, bias=zero_sb, scale=ESC)
                        if diag:
                            sl = pt[:, :, ws:ws + 2 * P]
                            nc.vector.tensor_mul(out=sl, in0=sl, in1=mask_sb)
                        pts.append(pt)
                    return pts

                def emit_av(qs, hp, kt0, pts, o_tiles, nkt):
                    for hh in range(2):
                        h = 2 * hp + hh
                        nc.tensor.matmul(
                            o_tiles[hh],
                            lhsT=vp[kt0 // 2][:, :,
                                              h * VW:h * VW + DH + 2],
                            rhs=pts[hh],
                            start=(kt0 == 0), stop=(kt0 + 2 == nkt),
                            perf_mode=DR, skip_group_check=True)

                def normalize(qs, hp, o_tiles):
                    for hh in range(2):
                        h = 2 * hp + hh
                        ops = o_tiles[hh]
                        den = nrm.tile([1, QS], f32, tag="den")
                        nc.vector.tensor_copy(out=den, in_=ops[DH:DH + 1, :])
                        dbc = nrm.tile([DH, QS], f32, tag="dbc")
                        nc.vector.reciprocal_approx_fast(
                            out=dbc[0:1, :], in_=den)
                        nc.gpsimd.partition_broadcast(dbc, dbc[0:1, :])
                        nc.vector.tensor_mul(
                            out=oT[h // 4][(h % 2) * DH:(h % 2) * DH + DH,
                                           (h // 2) % 2,
                                           qs * QS:(qs + 1) * QS],
                            in0=ops[0:DH, :], in1=dbc)

                # ---------- prologue compute ----------
                for st in range(2 * KT_PER_QS):
                    ln_compute(st)
                # chunk 0 prep runs inline; chunk 1 prep queued with deadline
                for st in range(KT_PER_QS):
                    ln_transpose(st)
                    v_proj(st)
                for (wT, dst, bias) in ((wqT, qT, bq_sb), (wkT, kT, bk_sb)):
                    for t in range(NOT):
                        qk_proj(0, wT, dst, bias, t)
                push_prep(1)

                # ---------- pipelined attention ----------
                slots = []
                for c in range(NQS):
                    nkt = (c + 1) * KT_PER_QS
                    for hp in range(HL // 2):
                        for kt0 in range(0, nkt, 2):
                            slots.append((c, hp, kt0, nkt))

                pending = None       # (qs, hp, kt0, pts, o_tiles, nkt, last)
                o_cur = None
                cur_chunk = -1
                for (c, hp, kt0, nkt) in slots:
                    if c != cur_chunk:
                        cur_chunk = c
                        if c > 0:
                            drain_deadline(c)
                        if c + 2 < NQS:
                            for st in range((c + 2) * KT_PER_QS,
                                            (c + 3) * KT_PER_QS):
                                ln_compute(st)
                            push_prep(c + 2)
                    if kt0 == 0:
                        o_cur = [op.tile([DH + 2, QS], f32, tag="o_ps",
                                         name="o_ps0"),
                                 op.tile([DH + 2, QS], f32, tag="o_ps",
                                         name="o_ps1")]
                    s_tiles = emit_qk(c, hp, kt0)
                    pts = emit_exp(c, kt0, s_tiles)
                    # pace helper work between QK and the pending AV
                    diag_r2 = kt0 - c * KT_PER_QS == 2
                    budget[0] += 330.0 if diag_r2 else 1180.0
                    pop_paced()
                    if pending is not None:
                        pq, php, pkt0, ppts, po, pnkt, plast = pending
                        emit_av(pq, php, pkt0, ppts, po, pnkt)
                        if plast:
                            normalize(pq, php, po)
                            if php == HL // 2 - 1:
                                push_outproj(pq)
                    pending = (c, hp, kt0, pts, o_cur, nkt,
                               kt0 + 2 == nkt)
                # flush the last slot
                pq, php, pkt0, ppts, po, pnkt, plast = pending
                emit_av(pq, php, pkt0, ppts, po, pnkt)
                normalize(pq, php, po)
                push_outproj(NQS - 1)

                # epilogue: drain all remaining helper work
                for (_, _, fn) in queue:
                    fn()

    nc.compile()
    return nc


def _get_nc():
    if "nc" not in _CACHE:
        _CACHE["nc"] = _build_nc()
    return _CACHE["nc"]


def make_in_maps(x, key_val_lengths, Wq, bq, Wkv, bkv, Wo, bo, ln_g, ln_b):
    import ml_dtypes
    fp8 = ml_dtypes.float8_e4m3
    bf16 = ml_dtypes.bfloat16

    x = np.ascontiguousarray(np.asarray(x, dtype=np.float32))
    lens = np.asarray(key_val_lengths).astype(np.int64)
    Wq = np.asarray(Wq, dtype=np.float32)
    Wkv = np.asarray(Wkv, dtype=np.float32)
    Wo = np.asarray(Wo, dtype=np.float32)
    bq = np.asarray(bq, dtype=np.float32)
    bkv = np.asarray(bkv, dtype=np.float32)
    ln_g = np.asarray(ln_g, dtype=np.float32)
    ln_b = np.asarray(ln_b, dtype=np.float32)

    # fold LayerNorm gain into the projection weights and shift into the
    # biases (exact algebra): q = ((x-mu)rstd*g + b) @ Wq.T + bq
    #                           = xhat @ (Wq*g).T + (Wq@b + bq)
    g64 = ln_g.astype(np.float64)
    b64 = ln_b.astype(np.float64)
    Wq64 = Wq.astype(np.float64) * g64
    Wk64 = Wkv[:D].astype(np.float64) * g64
    Wv64 = Wkv[D:].astype(np.float64) * g64
    bq_f = (Wq.astype(np.float64) @ b64 + bq).astype(np.float32)
    bk_f = (Wkv[:D].astype(np.float64) @ b64 + bkv[:D]).astype(np.float32)
    bv_f = Wv64 @ b64 + bkv[D:]          # folded into bo on host (f64)

    def pair_rows(wT, width):
        # [D or OH, width] fp8 -> [rows/2, 2, width] with 128-row pairs
        # interleaved for the DoubleRow contraction layout
        n = wT.shape[0]
        return np.ascontiguousarray(
            wT.reshape(n // 256, 2, P, width).transpose(0, 2, 1, 3)
            .reshape(n // 2, 2, width))

    in_maps = []
    for core in range(8):
        b, half = divmod(core, 2)
        sl = slice(half * OH, (half + 1) * OH)
        pad01 = (np.arange(S) < lens[b]).astype(np.float32)
        in_maps.append({
            "x": (x[b] * 0.5).astype(bf16),
            "wqt": pair_rows((Wq64[sl].T * 16.0).astype(fp8), OH),
            "wkt": pair_rows((Wk64[sl].T * 16.0).astype(fp8), OH),
            "wvt": pair_rows((Wv64[sl].T * 16.0).astype(fp8), OH),
            "wot": pair_rows((Wo[:, sl].T * 16.0).astype(fp8), D),
            "bq": np.ascontiguousarray((16.0 * bq_f[sl]).reshape(NOT, P).T),
            "bk": np.ascontiguousarray((16.0 * bk_f[sl]).reshape(NOT, P).T),
            "pad01": np.ascontiguousarray(pad01.reshape(NST, P).T),
        })
    bo_eff = (np.asarray(bo, np.float64)
              + np.asarray(Wo, np.float64) @ bv_f).astype(np.float32)
    return in_maps, bo_eff


def kernel(**inputs):
    from concourse.bass_utils import run_bass_kernel_spmd

    trace = bool(os.environ.get("KERNEL_TRACE"))
    if trace:
        try:
            import antenv.axon_hooks  # noqa: F401  (profiling shim present?)
        except ImportError:
            trace = False
    nc = _get_nc()
    in_maps, bo_eff = make_in_maps(**inputs)
    res = run_bass_kernel_spmd(
        nc, in_maps, core_ids=list(range(8)), trace=trace)
    _CACHE["last_results"] = res
    y = np.empty((B, S, D), dtype=np.float32)
    for b in range(B):
        y[b] = res.results[2 * b]["out"] + res.results[2 * b + 1]["out"] + bo_eff
    return y


# revision 15
# speedup vs baseline: 1.1309x; 1.0473x over previous
"""Trainium2 Bass kernel: causal multi-head self-attention block (pre-LN).

Full module computed on 8 NeuronCores:
    xn = LayerNorm(x); q = xn@Wq.T+bq; k,v = xn@Wkv.T+bkv
    out = softmax(mask(q k^T / sqrt(dh))) v @ Wo.T + bo + x
Sharding: core = batch_index * 2 + head_half.  Each core handles one batch
element and 8 of the 16 heads (column-parallel QKV, row-parallel Wo), emits a
partial [S, D] output including half the residual; host sums core pairs and
adds bo (with the V-bias contribution folded in on host: exact algebra).

v4: single ACT table set (rstd = exp(-0.5 ln(var+eps))); one wide ACTIVATE per
[P,2,QS] score tile, windowed past the fully-masked half of diagonal pairs;
causal mask as a static-fp8-tile multiply (r2 pairs draw from a zero-
initialized pt pool so the skipped window stays 0); x resident in SBUF as
bf16; software-pipelined slots (QK of slot i+1 -> paced helper work -> AV of
slot i) so the PE always has independent queued work while Scalar streams exp.
"""

import os
import sys

import numpy as np

sys.path.insert(0, "/opt/trn_rl_repo")

B, S, D, H = 4, 2048, 1024, 16
DH = D // H            # 64
HL = H // 2            # heads per core: 8
OH = HL * DH           # per-core head features: 512
EPS = 1e-5
P = 128                # SBUF partitions
NST = S // P           # 16 s-tiles
NFT = D // P           # 8 feature tiles
NFP = NFT // 2         # 4 paired feature tiles (DoubleRow)
NOT = OH // P          # 4 o-tiles (per-core head features)
NOP = NOT // 2         # 2 paired o-tiles
NVP = NST // 2         # 8 paired v key-tiles
QS = 512               # query super-tile (matmul moving free dim)
NQS = S // QS          # 4
KT_PER_QS = QS // P    # 4 k-tiles per q-super
VW = 80                # per-head stride in the padded V tiles (16B-aligned)
ESC = 0.125 / 256.0    # exp scale: 1/sqrt(DH) plus 1/(16*16) weight descale

_CACHE = {}


def _build_nc():
    import concourse.bass as bass
    import concourse.bacc as bacc
    import concourse.tile as tile
    from concourse import mybir

    f32 = mybir.dt.float32
    bf16 = mybir.dt.bfloat16
    fp8 = mybir.dt.float8e4
    Alu = mybir.AluOpType
    Act = mybir.ActivationFunctionType
    DR = mybir.MatmulPerfMode.DoubleRow

    nc = bacc.Bacc("TRN2", target_bir_lowering=False, debug=False, num_devices=8)

    # ---- DRAM I/O (per-core shard shapes; weights pre-paired + fp8, x16) ----
    x_d = nc.dram_tensor("x", [S, D], bf16, kind="ExternalInput").ap()  # x/2
    wq_d = nc.dram_tensor("wqt", [NFP * P, 2, OH], fp8, kind="ExternalInput").ap()
    wk_d = nc.dram_tensor("wkt", [NFP * P, 2, OH], fp8, kind="ExternalInput").ap()
    wv_d = nc.dram_tensor("wvt", [NFP * P, 2, OH], fp8, kind="ExternalInput").ap()
    wo_d = nc.dram_tensor("wot", [NOP * P, 2, D], fp8, kind="ExternalInput").ap()
    bq_d = nc.dram_tensor("bq", [P, NOT], f32, kind="ExternalInput").ap()   # x16
    bk_d = nc.dram_tensor("bk", [P, NOT], f32, kind="ExternalInput").ap()   # x16
    pad_d = nc.dram_tensor("pad01", [P, NST], f32, kind="ExternalInput").ap()
    out_d = nc.dram_tensor("out", [S, D], f32, kind="ExternalOutput").ap()

    def headview(ap_2d, stride, width):
        # [P, *] AP -> [P, HL, width] view with per-head stride
        return bass.AP(tensor=ap_2d.tensor, offset=ap_2d.offset,
                       ap=[ap_2d.ap[0], [stride, HL], [1, width]])

    with tile.TileContext(nc) as tc:
        with (
            tc.tile_pool(name="res", bufs=1) as res,       # resident tensors
            tc.tile_pool(name="small", bufs=4) as small,
        ):
            # ---------- constants ----------
            pad_sb = res.tile([P, NST], f32, tag="pad_sb")
            zero_sb = res.tile([P, 1], f32, tag="zero_sb")
            nc.vector.memset(zero_sb, 0.0)
            ident_b = res.tile([P, P], bf16, tag="ident_b")
            nc.gpsimd.memset(ident_b, 0.0)
            nc.gpsimd.affine_select(
                out=ident_b, in_=ident_b, compare_op=Alu.not_equal, fill=1.0,
                base=0, pattern=[[-1, P]], channel_multiplier=1)
            bq_sb = res.tile([P, NOT], f32, tag="bq_sb")
            bk_sb = res.tile([P, NOT], f32, tag="bk_sb")
            eps_sb = res.tile([P, 1], f32, tag="eps_sb")
            nc.vector.memset(eps_sb, EPS / 4.0)   # x/2 quarters the variance
            rstd_all = res.tile([P, NST], f32, tag="rstd_all")
            mb_all = res.tile([P, NST], f32, tag="mb_all")
            lnv_all = res.tile([P, NST], f32, tag="lnv_all")

            # ---------- resident big tensors ----------
            x_res = [res.tile([P, D], bf16, tag=f"xr{st}", name=f"xr{st}")
                     for st in range(NST)]
            xnT = [res.tile([P, 2, S], fp8, tag=f"xnT{j}", name=f"xnT{j}")
                   for j in range(NFP)]
            qT = [res.tile([P, S], bf16, tag=f"qT{t}", name=f"qT{t}")
                  for t in range(NOT)]
            kT = [res.tile([P, S], bf16, tag=f"kT{t}", name=f"kT{t}")
                  for t in range(NOT)]
            # V pairs augmented with a pad column per head at h*VW+64;
            # VW=80 keeps the DoubleRow ldweights 16B-aligned
            vp = [res.tile([P, 2, HL * VW], fp8, tag=f"vp{i}",
                           name=f"vp{i}") for i in range(NVP)]
            oT = [res.tile([P, 2, S], fp8, tag=f"oT{t}", name=f"oT{t}")
                  for t in range(NOP)]
            wqT = [res.tile([P, 2, OH], fp8, tag=f"wqT{j}", name=f"wqT{j}")
                   for j in range(NFP)]
            wkT = [res.tile([P, 2, OH], fp8, tag=f"wkT{j}", name=f"wkT{j}")
                   for j in range(NFP)]
            wvT = [res.tile([P, 2, OH], fp8, tag=f"wvT{j}", name=f"wvT{j}")
                   for j in range(NFP)]
            woT = [res.tile([P, 2, D], fp8, tag=f"woT{t}", name=f"woT{t}")
                   for t in range(NOP)]

            with (
                tc.tile_pool(name="pj_psum", bufs=2, space="PSUM") as pp,
                tc.tile_pool(name="s_psum", bufs=2, space="PSUM") as sp,
                tc.tile_pool(name="o_psum", bufs=2, space="PSUM") as op,
                tc.tile_pool(name="pt", bufs=4) as ptp,
                tc.tile_pool(name="pt2", bufs=2) as pt2p,
                tc.tile_pool(name="nrm", bufs=2) as nrm,
                tc.tile_pool(name="tmp", bufs=3) as tmp,
                tc.tile_pool(name="ye", bufs=3) as yp,
            ):
                # ---------- prologue DMAs ----------
                nc.sync.dma_start(out=pad_sb, in_=pad_d)
                nc.sync.dma_start(out=bq_sb, in_=bq_d)
                nc.sync.dma_start(out=bk_sb, in_=bk_d)
                for st in range(8):
                    nc.sync.dma_start(out=x_res[st],
                                      in_=x_d[st * P:(st + 1) * P, :])
                for j in range(NFP):
                    nc.sync.dma_start(out=wqT[j], in_=wq_d[j * P:(j + 1) * P, :, :])
                    nc.sync.dma_start(out=wkT[j], in_=wk_d[j * P:(j + 1) * P, :, :])
                    nc.sync.dma_start(out=wvT[j], in_=wv_d[j * P:(j + 1) * P, :, :])
                for t in range(NOP):
                    nc.sync.dma_start(out=woT[t], in_=wo_d[t * P:(t + 1) * P, :, :])
                for st in range(8, NST):
                    nc.sync.dma_start(out=x_res[st],
                                      in_=x_d[st * P:(st + 1) * P, :])

                # V tiles: 1.0 everywhere, then the per-head pad column gets
                # the 0/1 pad value for its two key tiles (once, up front)
                for i in range(NVP):
                    nc.gpsimd.memset(vp[i], 1.0)
                for st in range(NST):
                    vsl = vp[st // 2][:, st % 2, :]
                    ones = bass.AP(tensor=vsl.tensor, offset=vsl.offset + DH,
                                   ap=[vsl.ap[0], [VW, HL], [1, 1]])
                    nc.gpsimd.tensor_scalar_mul(
                        out=ones, in0=ones, scalar1=pad_sb[:, st:st + 1])

                # pre-zero the diagonal-r2 pt buffers: their [0, 2P) window
                # is never written, so AV reads zeros there forever
                for _ in range(2):
                    z = pt2p.tile([P, 2, QS], fp8, tag="pt2")
                    nc.vector.memset(z, 0.0)

                xns = {}

                def ln_stats(st):
                    # prologue-batched: all Ln ACTs come before all Exp ACTs
                    # so the ACT table set loads exactly once per function
                    stats = small.tile([P, 2, 6], f32, tag="stats")
                    for sg in range(2):
                        nc.vector.bn_stats(
                            out=stats[:, sg, :],
                            in_=x_res[st][:, sg * 512:(sg + 1) * 512])
                    mv = small.tile([P, 2], f32, tag="mv", bufs=8)
                    nc.vector.bn_aggr(out=mv, in_=stats)
                    nc.vector.tensor_scalar_max(
                        out=mv[:, 1:2], in0=mv[:, 1:2], scalar1=0.0)
                    return mv

                def ln_rstd(st, mv):
                    nc.scalar.activation(out=lnv_all[:, st:st + 1],
                                         in_=mv[:, 1:2],
                                         func=Act.Ln, bias=eps_sb, scale=1.0)
                    nc.vector.tensor_scalar(
                        out=mb_all[:, st:st + 1], in0=mv[:, 0:1],
                        scalar1=-1.0, scalar2=None, op0=Alu.mult)

                def ln_compute(st):
                    # rstd = exp(-0.5*ln(var+eps)); mb = -mean*rstd
                    xn = tmp.tile([P, D], bf16, tag="xn", bufs=9)
                    nc.vector.tensor_scalar(
                        out=xn, in0=x_res[st],
                        scalar1=rstd_all[:, st:st + 1],
                        scalar2=mb_all[:, st:st + 1],
                        op0=Alu.mult, op1=Alu.add)
                    xns[st] = xn

                def ln_transpose(st):
                    xn = xns.pop(st)
                    for j in range(NFT):
                        ps = pp.tile([P, P], bf16, tag="pj")
                        nc.tensor.transpose(
                            ps, xn[:, j * P:(j + 1) * P], ident_b)
                        nc.vector.tensor_copy(
                            out=xnT[j // 2][:, j % 2, st * P:(st + 1) * P],
                            in_=ps)

                def qk_proj(c, wT, dst, bias, t):
                    ps = pp.tile([P, QS], f32, tag="pj")
                    for j in range(NFP):
                        nc.tensor.matmul(
                            ps,
                            lhsT=wT[j][:, :, t * P:(t + 1) * P],
                            rhs=xnT[j][:, :, c * QS:(c + 1) * QS],
                            start=(j == 0), stop=(j == NFP - 1),
                            perf_mode=DR)
                    nc.vector.tensor_scalar_add(
                        out=dst[t][:, c * QS:(c + 1) * QS],
                        in0=ps, scalar1=bias[:, t:t + 1])

                def v_proj(st):
                    ps = pp.tile([P, OH], f32, tag="pj")
                    for j in range(NFP):
                        nc.tensor.matmul(
                            ps,
                            lhsT=xnT[j][:, :, st * P:(st + 1) * P],
                            rhs=wvT[j],
                            start=(j == 0), stop=(j == NFP - 1),
                            perf_mode=DR)
                    vsl = vp[st // 2][:, st % 2, :]
                    # v = raw*0.0625*pad (vb folded into bo on host)
                    nc.vector.tensor_scalar(
                        out=headview(vsl, VW, DH),
                        in0=headview(ps[:, :], DH, DH),
                        scalar1=0.0625, scalar2=pad_sb[:, st:st + 1],
                        op0=Alu.mult, op1=Alu.mult)

                def outproj(st, mc):
                    ps = pp.tile([P, QS], f32, tag="pj")
                    for ot in range(NOP):
                        nc.tensor.matmul(
                            ps,
                            lhsT=oT[ot][:, :, st * P:(st + 1) * P],
                            rhs=woT[ot][:, :, mc * QS:(mc + 1) * QS],
                            start=(ot == 0), stop=(ot == NOP - 1),
                            perf_mode=DR)
                    y_sb = yp.tile([P, QS], f32, tag="y_sb")
                    nc.vector.scalar_tensor_tensor(
                        out=y_sb, in0=ps, scalar=0.0625,
                        in1=x_res[st][:, mc * QS:(mc + 1) * QS],
                        op0=Alu.mult, op1=Alu.add)
                    nc.sync.dma_start(
                        out=out_d[st * P:(st + 1) * P,
                                  mc * QS:(mc + 1) * QS],
                        in_=y_sb)

                # ---------- helper work queue (PE filler) ----------
                # entries: (deadline_chunk, pe_cost_ns, fn)
                queue = []
                popped = [0.0]
                budget = [0.0]

                def push_prep(c):
                    c0 = c * KT_PER_QS
                    for st in range(c0, c0 + KT_PER_QS):
                        queue.append((c, 2200.0, lambda st=st: ln_transpose(st)))
                        queue.append((c, 964.0, lambda st=st: v_proj(st)))
                    for (wT, dst, bias) in ((wqT, qT, bq_sb),
                                            (wkT, kT, bk_sb)):
                        for t in range(NOT):
                            queue.append(
                                (c, 964.0,
                                 lambda wT=wT, dst=dst, bias=bias, t=t, c=c:
                                 qk_proj(c, wT, dst, bias, t)))

                def push_outproj(c):
                    for st in range(c * KT_PER_QS, (c + 1) * KT_PER_QS):
                        for mc in range(2):
                            queue.append(
                                (99, 600.0,
                                 lambda st=st, mc=mc: outproj(st, mc)))

                def pop_paced():
                    while queue and popped[0] < budget[0]:
                        _, cost, fn = queue.pop(0)
                        fn()
                        popped[0] += cost

                def drain_deadline(c):
                    keep = []
                    for ent in queue:
                        if ent[0] <= c:
                            ent[2]()
                            popped[0] += ent[1]
                        else:
                            keep.append(ent)
                    queue[:] = keep

                # ---------- attention slot machinery ----------
                def emit_qk(qs, hp, kt0):
                    # 4 QK matmuls; h0 rows 0-63 and h1 rows 64-127 run
                    # row-tile concurrent into separate PSUM tiles
                    tiles = []
                    for hh in range(2):
                        h = 2 * hp + hh
                        hq = qT[h // 2][(h % 2) * DH:(h % 2) * DH + DH, :]
                        hk = kT[h // 2][(h % 2) * DH:(h % 2) * DH + DH, :]
                        s_t = sp.tile([P, 2, QS], f32, tag="s_ps")
                        for i in range(2):
                            kt = kt0 + i
                            nc.tensor.matmul(
                                s_t[:, i, :],
                                lhsT=hk[:, kt * P:(kt + 1) * P],
                                rhs=hq[:, qs * QS:(qs + 1) * QS],
                                start=True, stop=True, skip_group_check=True)
                        tiles.append(s_t)
                    return tiles

                def emit_exp(qs, kt0, s_tiles):
                    diag = kt0 >= qs * KT_PER_QS
                    r2 = diag and (kt0 - qs * KT_PER_QS) == 2
                    ws = 2 * P if r2 else 0
                    pts = []
                    for s_t in s_tiles:
                        pool = pt2p if r2 else ptp
                        pt = pool.tile([P, 2, QS], fp8,
                                       tag="pt2" if r2 else "pt")
                        if r2:
                            nc.vector.memset(pt[:, :, :ws], 0.0)
                        nc.scalar.activation(
                            out=pt[:, :, ws:], in_=s_t[:, :, ws:],
                            func=Act.Exp, bias=zero_sb, scale=ESC)
                        if diag:
                            nc.gpsimd.affine_select(
                                out=pt[:, :, ws:ws + 2 * P],
                                in_=pt[:, :, ws:ws + 2 * P],
                                compare_op=Alu.is_ge, fill=0.0, base=0,
                                pattern=[[-P, 2], [1, 2 * P]],
                                channel_multiplier=-1)
                        pts.append(pt)
                    return pts

                def emit_av(qs, hp, kt0, pts, o_tiles, nkt):
                    for hh in range(2):
                        h = 2 * hp + hh
                        nc.tensor.matmul(
                            o_tiles[hh],
                            lhsT=vp[kt0 // 2][:, :,
                                              h * VW:h * VW + DH + 2],
                            rhs=pts[hh],
                            start=(kt0 == 0), stop=(kt0 + 2 == nkt),
                            perf_mode=DR, skip_group_check=True)

                def normalize(qs, hp, o_tiles):
                    for hh in range(2):
                        h = 2 * hp + hh
                        ops = o_tiles[hh]
                        # reciprocal_approx_fast misreads PSUM inputs; stage
                        # den through SBUF on the Scalar engine (Copy is in
                        # every ACT table set, so no table swap)
                        den = nrm.tile([1, QS], f32, tag="den")
                        nc.scalar.copy(out=den, in_=ops[DH:DH + 1, :])
                        dbc = nrm.tile([DH, QS], f32, tag="dbc")
                        nc.vector.reciprocal_approx_fast(
                            out=dbc[0:1, :], in_=den)
                        nc.gpsimd.partition_broadcast(dbc, dbc[0:1, :])
                        nc.vector.tensor_mul(
                            out=oT[h // 4][(h % 2) * DH:(h % 2) * DH + DH,
                                           (h // 2) % 2,
                                           qs * QS:(qs + 1) * QS],
                            in0=ops[0:DH, :], in1=dbc)

                # ---------- prologue compute ----------
                for st in range(NST):
                    mv = ln_stats(st)
                    ln_rstd(st, mv)
                nc.scalar.activation(out=rstd_all, in_=lnv_all,
                                     func=Act.Exp, bias=zero_sb, scale=-0.5)
                nc.vector.tensor_mul(out=mb_all, in0=mb_all, in1=rstd_all)
                for st in range(2 * KT_PER_QS):
                    ln_compute(st)
                # chunk 0 prep runs inline; chunk 1 prep queued with deadline
                for st in range(KT_PER_QS):
                    ln_transpose(st)
                    v_proj(st)
                for (wT, dst, bias) in ((wqT, qT, bq_sb), (wkT, kT, bk_sb)):
                    for t in range(NOT):
                        qk_proj(0, wT, dst, bias, t)
                push_prep(1)

                # ---------- pipelined attention ----------
                slots = []
                for c in range(NQS):
                    nkt = (c + 1) * KT_PER_QS
                    for hp in range(HL // 2):
                        for kt0 in range(0, nkt, 2):
                            slots.append((c, hp, kt0, nkt))

                pending = None       # (qs, hp, kt0, pts, o_tiles, nkt, last)
                o_cur = None
                cur_chunk = -1
                for (c, hp, kt0, nkt) in slots:
                    if c != cur_chunk:
                        cur_chunk = c
                        if c > 0:
                            drain_deadline(c)
                        if c + 2 < NQS:
                            for st in range((c + 2) * KT_PER_QS,
                                            (c + 3) * KT_PER_QS):
                                ln_compute(st)
                            push_prep(c + 2)
                    if kt0 == 0:
                        o_cur = [op.tile([DH + 2, QS], f32, tag="o_ps",
                                         name="o_ps0"),
                                 op.tile([DH + 2, QS], f32, tag="o_ps",
                                         name="o_ps1")]
                    s_tiles = emit_qk(c, hp, kt0)
                    pts = emit_exp(c, kt0, s_tiles)
                    # pace helper work between QK and the pending AV
                    diag_r2 = kt0 - c * KT_PER_QS == 2
                    budget[0] += 330.0 if diag_r2 else 1180.0
                    pop_paced()
                    if pending is not None:
                        pq, php, pkt0, ppts, po, pnkt, plast = pending
                        emit_av(pq, php, pkt0, ppts, po, pnkt)
                        if plast:
                            normalize(pq, php, po)
                            if php == HL // 2 - 1:
                                push_outproj(pq)
                    pending = (c, hp, kt0, pts, o_cur, nkt,
                               kt0 + 2 == nkt)
                # flush the last slot
                pq, php, pkt0, ppts, po, pnkt, plast = pending
                emit_av(pq, php, pkt0, ppts, po, pnkt)
                normalize(pq, php, po)
                push_outproj(NQS - 1)

                # epilogue: drain all remaining helper work
                for (_, _, fn) in queue:
                    fn()

    nc.compile()
    return nc


def _get_nc():
    if "nc" not in _CACHE:
        _CACHE["nc"] = _build_nc()
    return _CACHE["nc"]


def make_in_maps(x, key_val_lengths, Wq, bq, Wkv, bkv, Wo, bo, ln_g, ln_b):
    import ml_dtypes
    fp8 = ml_dtypes.float8_e4m3
    bf16 = ml_dtypes.bfloat16

    x = np.ascontiguousarray(np.asarray(x, dtype=np.float32))
    lens = np.asarray(key_val_lengths).astype(np.int64)
    Wq = np.asarray(Wq, dtype=np.float32)
    Wkv = np.asarray(Wkv, dtype=np.float32)
    Wo = np.asarray(Wo, dtype=np.float32)
    bq = np.asarray(bq, dtype=np.float32)
    bkv = np.asarray(bkv, dtype=np.float32)
    ln_g = np.asarray(ln_g, dtype=np.float32)
    ln_b = np.asarray(ln_b, dtype=np.float32)

    # fold LayerNorm gain into the projection weights and shift into the
    # biases (exact algebra): q = ((x-mu)rstd*g + b) @ Wq.T + bq
    #                           = xhat @ (Wq*g).T + (Wq@b + bq)
    g64 = ln_g.astype(np.float64)
    b64 = ln_b.astype(np.float64)
    Wq64 = Wq.astype(np.float64) * g64
    Wk64 = Wkv[:D].astype(np.float64) * g64
    Wv64 = Wkv[D:].astype(np.float64) * g64
    bq_f = (Wq.astype(np.float64) @ b64 + bq).astype(np.float32)
    bk_f = (Wkv[:D].astype(np.float64) @ b64 + bkv[:D]).astype(np.float32)
    bv_f = Wv64 @ b64 + bkv[D:]          # folded into bo on host (f64)

    def pair_rows(wT, width):
        # [D or OH, width] fp8 -> [rows/2, 2, width] with 128-row pairs
        # interleaved for the DoubleRow contraction layout
        n = wT.shape[0]
        return np.ascontiguousarray(
            wT.reshape(n // 256, 2, P, width).transpose(0, 2, 1, 3)
            .reshape(n // 2, 2, width))

    in_maps = []
    for core in range(8):
        b, half = divmod(core, 2)
        sl = slice(half * OH, (half + 1) * OH)
        pad01 = (np.arange(S) < lens[b]).astype(np.float32)
        in_maps.append({
            "x": (x[b] * 0.5).astype(bf16),
            "wqt": pair_rows((Wq64[sl].T * 16.0).astype(fp8), OH),
            "wkt": pair_rows((Wk64[sl].T * 16.0).astype(fp8), OH),
            "wvt": pair_rows((Wv64[sl].T * 16.0).astype(fp8), OH),
            "wot": pair_rows((Wo[:, sl].T * 16.0).astype(fp8), D),
            "bq": np.ascontiguousarray((16.0 * bq_f[sl]).reshape(NOT, P).T),
            "bk": np.ascontiguousarray((16.0 * bk_f[sl]).reshape(NOT, P).T),
            "pad01": np.ascontiguousarray(pad01.reshape(NST, P).T),
        })
    bo_eff = (np.asarray(bo, np.float64)
              + np.asarray(Wo, np.float64) @ bv_f).astype(np.float32)
    return in_maps, bo_eff


def kernel(**inputs):
    from concourse.bass_utils import run_bass_kernel_spmd

    trace = bool(os.environ.get("KERNEL_TRACE"))
    if trace:
        try:
            import antenv.axon_hooks  # noqa: F401  (profiling shim present?)
        except ImportError:
            trace = False
    nc = _get_nc()
    in_maps, bo_eff = make_in_maps(**inputs)
    res = run_bass_kernel_spmd(
        nc, in_maps, core_ids=list(range(8)), trace=trace)
    _CACHE["last_results"] = res
    y = np.empty((B, S, D), dtype=np.float32)
    for b in range(B):
        y[b] = res.results[2 * b]["out"] + res.results[2 * b + 1]["out"] + bo_eff
    return y


# revision 21
# speedup vs baseline: 1.1903x; 1.0525x over previous
"""Trainium2 Bass kernel: causal multi-head self-attention block (pre-LN).

Full module computed on 8 NeuronCores:
    xn = LayerNorm(x); q = xn@Wq.T+bq; k,v = xn@Wkv.T+bkv
    out = softmax(mask(q k^T / sqrt(dh))) v @ Wo.T + bo + x
Sharding: core = batch_index * 2 + head_half.  Each core handles one batch
element and 8 of the 16 heads (column-parallel QKV, row-parallel Wo), emits a
partial [S, D] output including half the residual; host sums core pairs and
adds bo (with the V-bias contribution folded in on host: exact algebra).

v4: single ACT table set (rstd = exp(-0.5 ln(var+eps))); one wide ACTIVATE per
[P,2,QS] score tile, windowed past the fully-masked half of diagonal pairs;
causal mask as a static-fp8-tile multiply (r2 pairs draw from a zero-
initialized pt pool so the skipped window stays 0); x resident in SBUF as
bf16; software-pipelined slots (QK of slot i+1 -> paced helper work -> AV of
slot i) so the PE always has independent queued work while Scalar streams exp.
"""

import os
import sys

import numpy as np

sys.path.insert(0, "/opt/trn_rl_repo")

B, S, D, H = 4, 2048, 1024, 16
DH = D // H            # 64
HL = H // 2            # heads per core: 8
OH = HL * DH           # per-core head features: 512
EPS = 1e-5
P = 128                # SBUF partitions
NST = S // P           # 16 s-tiles
NFT = D // P           # 8 feature tiles
NFP = NFT // 2         # 4 paired feature tiles (DoubleRow)
NOT = OH // P          # 4 o-tiles (per-core head features)
NOP = NOT // 2         # 2 paired o-tiles
NVP = NST // 2         # 8 paired v key-tiles
QS = 512               # query super-tile (matmul moving free dim)
NQS = S // QS          # 4
KT_PER_QS = QS // P    # 4 k-tiles per q-super
VW = 80                # per-head stride in the padded V tiles (16B-aligned)
ESC = 0.125 / 256.0    # exp scale: 1/sqrt(DH) plus 1/(16*16) weight descale

_CACHE = {}


def _build_nc():
    import concourse.bass as bass
    import concourse.bacc as bacc
    import concourse.tile as tile
    from concourse import mybir

    f32 = mybir.dt.float32
    bf16 = mybir.dt.bfloat16
    fp8 = mybir.dt.float8e4
    Alu = mybir.AluOpType
    Act = mybir.ActivationFunctionType
    DR = mybir.MatmulPerfMode.DoubleRow

    nc = bacc.Bacc("TRN2", target_bir_lowering=False, debug=False, num_devices=8)

    # ---- DRAM I/O (per-core shard shapes; weights pre-paired + fp8, x16) ----
    x_d = nc.dram_tensor("x", [S, D], bf16, kind="ExternalInput").ap()  # x/2
    wq_d = nc.dram_tensor("wqt", [NFP * P, 2, OH], fp8, kind="ExternalInput").ap()
    wk_d = nc.dram_tensor("wkt", [NFP * P, 2, OH], fp8, kind="ExternalInput").ap()
    wv_d = nc.dram_tensor("wvt", [NFP * P, 2, OH], fp8, kind="ExternalInput").ap()
    wo_d = nc.dram_tensor("wot", [NOP * P, 2, D], fp8, kind="ExternalInput").ap()
    bq_d = nc.dram_tensor("bq", [P, NOT], f32, kind="ExternalInput").ap()   # x16
    bk_d = nc.dram_tensor("bk", [P, NOT], f32, kind="ExternalInput").ap()   # x16
    pad_d = nc.dram_tensor("pad01", [P, NST], f32, kind="ExternalInput").ap()
    out_d = nc.dram_tensor("out", [S, D], f32, kind="ExternalOutput").ap()

    def headview(ap_2d, stride, width):
        # [P, *] AP -> [P, HL, width] view with per-head stride
        return bass.AP(tensor=ap_2d.tensor, offset=ap_2d.offset,
                       ap=[ap_2d.ap[0], [stride, HL], [1, width]])

    with tile.TileContext(nc) as tc:
        with (
            tc.tile_pool(name="res", bufs=1) as res,       # resident tensors
            tc.tile_pool(name="small", bufs=4) as small,
        ):
            # ---------- constants ----------
            pad_sb = res.tile([P, NST], f32, tag="pad_sb")
            zero_sb = res.tile([P, 1], f32, tag="zero_sb")
            nc.vector.memset(zero_sb, 0.0)
            ident_b = res.tile([P, P], bf16, tag="ident_b")
            nc.gpsimd.memset(ident_b, 0.0)
            nc.gpsimd.affine_select(
                out=ident_b, in_=ident_b, compare_op=Alu.not_equal, fill=1.0,
                base=0, pattern=[[-1, P]], channel_multiplier=1)
            bq_sb = res.tile([P, NOT], f32, tag="bq_sb")
            bk_sb = res.tile([P, NOT], f32, tag="bk_sb")
            rstd_all = res.tile([P, NST], f32, tag="rstd_all")
            mb_all = res.tile([P, NST], f32, tag="mb_all")
            mv_all = res.tile([P, 2, NST], f32, tag="mv_all")
            qk1 = res.tile([P, NST], f32, tag="qk1")   # quake-rsqrt scratch
            qk2 = res.tile([P, NST], f32, tag="qk2")

            # ---------- resident big tensors ----------
            x_res = [res.tile([P, D], bf16, tag=f"xr{st}", name=f"xr{st}")
                     for st in range(NST)]
            xnT = [res.tile([P, 2, S], fp8, tag=f"xnT{j}", name=f"xnT{j}")
                   for j in range(NFP)]
            qT = [res.tile([P, S], bf16, tag=f"qT{t}", name=f"qT{t}")
                  for t in range(NOT)]
            kT = [res.tile([P, S], bf16, tag=f"kT{t}", name=f"kT{t}")
                  for t in range(NOT)]
            # V pairs augmented with a pad column per head at h*VW+64;
            # VW=80 keeps the DoubleRow ldweights 16B-aligned
            vp = [res.tile([P, 2, HL * VW], fp8, tag=f"vp{i}",
                           name=f"vp{i}") for i in range(NVP)]
            oT = [res.tile([P, 2, S], fp8, tag=f"oT{t}", name=f"oT{t}")
                  for t in range(NOP)]
            wqT = [res.tile([P, 2, OH], fp8, tag=f"wqT{j}", name=f"wqT{j}")
                   for j in range(NFP)]
            wkT = [res.tile([P, 2, OH], fp8, tag=f"wkT{j}", name=f"wkT{j}")
                   for j in range(NFP)]
            wvT = [res.tile([P, 2, OH], fp8, tag=f"wvT{j}", name=f"wvT{j}")
                   for j in range(NFP)]
            woT = [res.tile([P, 2, D], fp8, tag=f"woT{t}", name=f"woT{t}")
                   for t in range(NOP)]

            with (
                tc.tile_pool(name="pj_psum", bufs=2, space="PSUM") as pp,
                tc.tile_pool(name="s_psum", bufs=2, space="PSUM") as sp,
                tc.tile_pool(name="o_psum", bufs=2, space="PSUM") as op,
                tc.tile_pool(name="pt", bufs=4) as ptp,
                tc.tile_pool(name="pt2", bufs=2) as pt2p,
                tc.tile_pool(name="nrm", bufs=2) as nrm,
                tc.tile_pool(name="tmp", bufs=3) as tmp,
                tc.tile_pool(name="ye", bufs=3) as yp,
            ):
                # ---------- prologue DMAs ----------
                nc.sync.dma_start(out=pad_sb, in_=pad_d)
                nc.sync.dma_start(out=bq_sb, in_=bq_d)
                nc.sync.dma_start(out=bk_sb, in_=bk_d)
                for st in range(8):
                    nc.sync.dma_start(out=x_res[st],
                                      in_=x_d[st * P:(st + 1) * P, :])
                for j in range(NFP):
                    nc.sync.dma_start(out=wqT[j], in_=wq_d[j * P:(j + 1) * P, :, :])
                    nc.sync.dma_start(out=wkT[j], in_=wk_d[j * P:(j + 1) * P, :, :])
                    nc.sync.dma_start(out=wvT[j], in_=wv_d[j * P:(j + 1) * P, :, :])
                for t in range(NOP):
                    nc.sync.dma_start(out=woT[t], in_=wo_d[t * P:(t + 1) * P, :, :])
                for st in range(8, NST):
                    nc.sync.dma_start(out=x_res[st],
                                      in_=x_d[st * P:(st + 1) * P, :])

                # V tiles: 1.0 everywhere, then the per-head pad column gets
                # the 0/1 pad value for its two key tiles (once, up front)
                for i in range(NVP):
                    nc.gpsimd.memset(vp[i], 1.0)
                for st in range(NST):
                    vsl = vp[st // 2][:, st % 2, :]
                    ones = bass.AP(tensor=vsl.tensor, offset=vsl.offset + DH,
                                   ap=[vsl.ap[0], [VW, HL], [1, 1]])
                    nc.gpsimd.tensor_scalar_mul(
                        out=ones, in0=ones, scalar1=pad_sb[:, st:st + 1])

                # pre-zero the diagonal-r2 pt buffers: their [0, 2P) window
                # is never written, so AV reads zeros there forever
                for _ in range(2):
                    z = pt2p.tile([P, 2, QS], fp8, tag="pt2")
                    nc.vector.memset(z, 0.0)

                xns = {}
                i32 = mybir.dt.int32

                def ln_stats(st):
                    stats = small.tile([P, 2, 6], f32, tag="stats")
                    for sg in range(2):
                        nc.vector.bn_stats(
                            out=stats[:, sg, :],
                            in_=x_res[st][:, sg * 512:(sg + 1) * 512])
                    nc.vector.bn_aggr(out=mv_all[:, :, st], in_=stats)

                def ln_rstd(lo, hi):
                    # rstd = 1/sqrt(var+eps) entirely on DVE for tiles
                    # [lo,hi): Quake bitcast seed + 2 Newton iterations
                    # (~5e-6 rel err).  No ScalarE table swap traffic.
                    v = qk1[:, lo:hi]
                    y = qk2[:, lo:hi]
                    mean = mv_all[:, 0, lo:hi]
                    var = mv_all[:, 1, lo:hi]
                    nc.vector.tensor_scalar(
                        out=v, in0=var, scalar1=0.0, scalar2=EPS / 4.0,
                        op0=Alu.max, op1=Alu.add)
                    nc.vector.tensor_scalar(
                        out=y.bitcast(i32), in0=v.bitcast(i32),
                        scalar1=1, scalar2=None,
                        op0=Alu.logical_shift_right)
                    nc.vector.tensor_scalar(
                        out=y.bitcast(i32), in0=y.bitcast(i32),
                        scalar1=-1, scalar2=0x5F3759DF,
                        op0=Alu.mult, op1=Alu.add)
                    r = rstd_all[:, lo:hi]
                    for _ in range(2):
                        t = mb_all[:, lo:hi]   # scratch before mb written
                        nc.vector.tensor_mul(out=t, in0=y, in1=y)
                        nc.vector.tensor_mul(out=t, in0=t, in1=v)
                        nc.vector.tensor_scalar(
                            out=t, in0=t, scalar1=-0.5, scalar2=1.5,
                            op0=Alu.mult, op1=Alu.add)
                        nc.vector.tensor_mul(out=y, in0=y, in1=t)
                    nc.vector.tensor_copy(out=r, in_=y)
                    # mb = -mean*rstd
                    nc.vector.tensor_scalar(
                        out=mb_all[:, lo:hi], in0=mean, scalar1=-1.0,
                        scalar2=None, op0=Alu.mult)
                    nc.vector.tensor_mul(out=mb_all[:, lo:hi],
                                         in0=mb_all[:, lo:hi], in1=r)

                def ln_compute(st):
                    xn = tmp.tile([P, D], bf16, tag="xn", bufs=9)
                    nc.vector.tensor_scalar(
                        out=xn, in0=x_res[st],
                        scalar1=rstd_all[:, st:st + 1],
                        scalar2=mb_all[:, st:st + 1],
                        op0=Alu.mult, op1=Alu.add)
                    xns[st] = xn

                def ln_transpose(st):
                    # 4 transposes share one PSUM tile; 2 paired copies out
                    xn = xns.pop(st)
                    for g in range(2):
                        ps = pp.tile([P, 4, P], bf16, tag="pj")
                        for m in range(4):
                            j = 4 * g + m
                            nc.tensor.transpose(
                                ps[:, m, :], xn[:, j * P:(j + 1) * P],
                                ident_b)
                        for h in range(2):
                            nc.vector.tensor_copy(
                                out=xnT[2 * g + h][:, :,
                                                   st * P:(st + 1) * P],
                                in_=ps[:, 2 * h:2 * h + 2, :])

                def qk_proj(c, wT, dst, bias, t):
                    ps = pp.tile([P, QS], f32, tag="pj")
                    for j in range(NFP):
                        nc.tensor.matmul(
                            ps,
                            lhsT=wT[j][:, :, t * P:(t + 1) * P],
                            rhs=xnT[j][:, :, c * QS:(c + 1) * QS],
                            start=(j == 0), stop=(j == NFP - 1),
                            perf_mode=DR)
                    nc.vector.tensor_scalar_add(
                        out=dst[t][:, c * QS:(c + 1) * QS],
                        in0=ps, scalar1=bias[:, t:t + 1])

                def v_proj(st):
                    ps = pp.tile([P, OH], f32, tag="pj")
                    for j in range(NFP):
                        nc.tensor.matmul(
                            ps,
                            lhsT=xnT[j][:, :, st * P:(st + 1) * P],
                            rhs=wvT[j],
                            start=(j == 0), stop=(j == NFP - 1),
                            perf_mode=DR)
                    vsl = vp[st // 2][:, st % 2, :]
                    # v = raw*0.0625*pad (vb folded into bo on host)
                    nc.vector.tensor_scalar(
                        out=headview(vsl, VW, DH),
                        in0=headview(ps[:, :], DH, DH),
                        scalar1=0.0625, scalar2=pad_sb[:, st:st + 1],
                        op0=Alu.mult, op1=Alu.mult)

                def outproj(st, mc):
                    ps = pp.tile([P, QS], f32, tag="pj")
                    for ot in range(NOP):
                        nc.tensor.matmul(
                            ps,
                            lhsT=oT[ot][:, :, st * P:(st + 1) * P],
                            rhs=woT[ot][:, :, mc * QS:(mc + 1) * QS],
                            start=(ot == 0), stop=(ot == NOP - 1),
                            perf_mode=DR)
                    y_sb = yp.tile([P, QS], f32, tag="y_sb")
                    nc.vector.scalar_tensor_tensor(
                        out=y_sb, in0=ps, scalar=0.0625,
                        in1=x_res[st][:, mc * QS:(mc + 1) * QS],
                        op0=Alu.mult, op1=Alu.add)
                    nc.sync.dma_start(
                        out=out_d[st * P:(st + 1) * P,
                                  mc * QS:(mc + 1) * QS],
                        in_=y_sb)

                # ---------- helper work queue (PE filler) ----------
                # entries: (deadline_chunk, pe_cost_ns, fn)
                queue = []
                popped = [0.0]
                budget = [0.0]

                def push_prep(c):
                    c0 = c * KT_PER_QS
                    for st in range(c0, c0 + KT_PER_QS):
                        queue.append((c, 2200.0, lambda st=st: ln_transpose(st)))
                        queue.append((c, 964.0, lambda st=st: v_proj(st)))
                    for (wT, dst, bias) in ((wqT, qT, bq_sb),
                                            (wkT, kT, bk_sb)):
                        for t in range(NOT):
                            queue.append(
                                (c, 964.0,
                                 lambda wT=wT, dst=dst, bias=bias, t=t, c=c:
                                 qk_proj(c, wT, dst, bias, t)))

                def push_outproj(c):
                    for st in range(c * KT_PER_QS, (c + 1) * KT_PER_QS):
                        for mc in range(2):
                            queue.append(
                                (99, 600.0,
                                 lambda st=st, mc=mc: outproj(st, mc)))

                def pop_paced():
                    while queue and popped[0] < budget[0]:
                        _, cost, fn = queue.pop(0)
                        fn()
                        popped[0] += cost

                def drain_deadline(c):
                    keep = []
                    for ent in queue:
                        if ent[0] <= c:
                            ent[2]()
                            popped[0] += ent[1]
                        else:
                            keep.append(ent)
                    queue[:] = keep

                # ---------- attention slot machinery ----------
                def emit_qk(qs, hp, kt0):
                    # 4 QK matmuls; h0 rows 0-63 and h1 rows 64-127 run
                    # row-tile concurrent into separate PSUM tiles
                    tiles = []
                    for hh in range(2):
                        h = 2 * hp + hh
                        hq = qT[h // 2][(h % 2) * DH:(h % 2) * DH + DH, :]
                        hk = kT[h // 2][(h % 2) * DH:(h % 2) * DH + DH, :]
                        s_t = sp.tile([P, 2, QS], f32, tag="s_ps")
                        for i in range(2):
                            kt = kt0 + i
                            nc.tensor.matmul(
                                s_t[:, i, :],
                                lhsT=hk[:, kt * P:(kt + 1) * P],
                                rhs=hq[:, qs * QS:(qs + 1) * QS],
                                start=True, stop=True, skip_group_check=True)
                        tiles.append(s_t)
                    return tiles

                def emit_exp(qs, kt0, s_tiles):
                    diag = kt0 >= qs * KT_PER_QS
                    r2 = diag and (kt0 - qs * KT_PER_QS) == 2
                    ws = 2 * P if r2 else 0
                    pts = []
                    for s_t in s_tiles:
                        pool = pt2p if r2 else ptp
                        pt = pool.tile([P, 2, QS], fp8,
                                       tag="pt2" if r2 else "pt")
                        if r2:
                            nc.vector.memset(pt[:, :, :ws], 0.0)
                        nc.scalar.activation(
                            out=pt[:, :, ws:], in_=s_t[:, :, ws:],
                            func=Act.Exp, bias=zero_sb, scale=ESC)
                        if diag:
                            nc.gpsimd.affine_select(
                                out=pt[:, :, ws:ws + 2 * P],
                                in_=pt[:, :, ws:ws + 2 * P],
                                compare_op=Alu.is_ge, fill=0.0, base=0,
                                pattern=[[-P, 2], [1, 2 * P]],
                                channel_multiplier=-1)
                        pts.append(pt)
                    return pts

                def emit_av(qs, hp, kt0, pts, o_tiles, nkt):
                    for hh in range(2):
                        h = 2 * hp + hh
                        nc.tensor.matmul(
                            o_tiles[hh],
                            lhsT=vp[kt0 // 2][:, :,
                                              h * VW:h * VW + DH + 2],
                            rhs=pts[hh],
                            start=(kt0 == 0), stop=(kt0 + 2 == nkt),
                            perf_mode=DR, skip_group_check=True)

                def normalize(qs, hp, o_tiles):
                    for hh in range(2):
                        h = 2 * hp + hh
                        ops = o_tiles[hh]
                        # reciprocal_approx_fast misreads PSUM inputs; stage
                        # den through SBUF on the Scalar engine (Copy is in
                        # every ACT table set, so no table swap)
                        den = nrm.tile([1, QS], f32, tag="den")
                        nc.scalar.copy(out=den, in_=ops[DH:DH + 1, :])
                        dbc = nrm.tile([DH, QS], f32, tag="dbc")
                        nc.vector.reciprocal_approx_fast(
                            out=dbc[0:1, :], in_=den)
                        nc.gpsimd.partition_broadcast(dbc, dbc[0:1, :])
                        nc.vector.tensor_mul(
                            out=oT[h // 4][(h % 2) * DH:(h % 2) * DH + DH,
                                           (h // 2) % 2,
                                           qs * QS:(qs + 1) * QS],
                            in0=ops[0:DH, :], in1=dbc)

                # ---------- prologue compute ----------
                # minimal critical path to the first transposes: stats for
                # tiles 0-3 only, then interleave the rest with PE work
                for st in range(4):
                    ln_stats(st)
                ln_rstd(0, 4)
                for st in range(4):
                    ln_compute(st)
                ln_transpose(0)
                v_proj(0)
                ln_transpose(1)
                v_proj(1)
                for st in range(4, 6):
                    ln_stats(st)
                ln_transpose(2)
                v_proj(2)
                for st in range(6, 8):
                    ln_stats(st)
                ln_transpose(3)
                v_proj(3)
                ln_rstd(4, 8)
                for (wT, dst, bias) in ((wqT, qT, bq_sb), (wkT, kT, bk_sb)):
                    for t in range(NOT):
                        qk_proj(0, wT, dst, bias, t)
                for st in range(4, 8):
                    ln_compute(st)
                push_prep(1)
                # stats + rstd for tiles 8-15 pop early in chunk 0 (DVE-only)
                for st0 in range(8, NST, 2):
                    queue.append((1, 0.0, lambda st0=st0: (
                        ln_stats(st0), ln_stats(st0 + 1))))
                queue.append((1, 0.0, lambda: ln_rstd(8, NST)))

                # ---------- pipelined attention ----------
                slots = []
                for c in range(NQS):
                    nkt = (c + 1) * KT_PER_QS
                    for hp in range(HL // 2):
                        for kt0 in range(0, nkt, 2):
                            slots.append((c, hp, kt0, nkt))

                pending = None       # (qs, hp, kt0, pts, o_tiles, nkt, last)
                o_cur = None
                cur_chunk = -1
                for (c, hp, kt0, nkt) in slots:
                    if c != cur_chunk:
                        cur_chunk = c
                        if c > 0:
                            drain_deadline(c)
                        if c + 2 < NQS:
                            for st in range((c + 2) * KT_PER_QS,
                                            (c + 3) * KT_PER_QS):
                                queue.append(
                                    (c + 1, 0.0,
                                     lambda st=st: ln_compute(st)))
                            push_prep(c + 2)
                    if kt0 == 0:
                        o_cur = [op.tile([DH + 2, QS], f32, tag="o_ps",
                                         name="o_ps0"),
                                 op.tile([DH + 2, QS], f32, tag="o_ps",
                                         name="o_ps1")]
                    s_tiles = emit_qk(c, hp, kt0)
                    pts = emit_exp(c, kt0, s_tiles)
                    # pace helper work between QK and the pending AV
                    diag_r2 = kt0 - c * KT_PER_QS == 2
                    budget[0] += 330.0 if diag_r2 else 1180.0
                    pop_paced()
                    if pending is not None:
                        pq, php, pkt0, ppts, po, pnkt, plast = pending
                        emit_av(pq, php, pkt0, ppts, po, pnkt)
                        if plast:
                            normalize(pq, php, po)
                            if php == HL // 2 - 1:
                                push_outproj(pq)
                    pending = (c, hp, kt0, pts, o_cur, nkt,
                               kt0 + 2 == nkt)
                # flush the last slot
                pq, php, pkt0, ppts, po, pnkt, plast = pending
                emit_av(pq, php, pkt0, ppts, po, pnkt)
                normalize(pq, php, po)
                push_outproj(NQS - 1)

                # epilogue: drain all remaining helper work
                for (_, _, fn) in queue:
                    fn()

    nc.compile()
    return nc


def _get_nc():
    if "nc" not in _CACHE:
        _CACHE["nc"] = _build_nc()
    return _CACHE["nc"]


def make_in_maps(x, key_val_lengths, Wq, bq, Wkv, bkv, Wo, bo, ln_g, ln_b):
    import ml_dtypes
    fp8 = ml_dtypes.float8_e4m3
    bf16 = ml_dtypes.bfloat16

    x = np.ascontiguousarray(np.asarray(x, dtype=np.float32))
    lens = np.asarray(key_val_lengths).astype(np.int64)
    Wq = np.asarray(Wq, dtype=np.float32)
    Wkv = np.asarray(Wkv, dtype=np.float32)
    Wo = np.asarray(Wo, dtype=np.float32)
    bq = np.asarray(bq, dtype=np.float32)
    bkv = np.asarray(bkv, dtype=np.float32)
    ln_g = np.asarray(ln_g, dtype=np.float32)
    ln_b = np.asarray(ln_b, dtype=np.float32)

    # fold LayerNorm gain into the projection weights and shift into the
    # biases (exact algebra): q = ((x-mu)rstd*g + b) @ Wq.T + bq
    #                           = xhat @ (Wq*g).T + (Wq@b + bq)
    g64 = ln_g.astype(np.float64)
    b64 = ln_b.astype(np.float64)
    Wq64 = Wq.astype(np.float64) * g64
    Wk64 = Wkv[:D].astype(np.float64) * g64
    Wv64 = Wkv[D:].astype(np.float64) * g64
    bq_f = (Wq.astype(np.float64) @ b64 + bq).astype(np.float32)
    bk_f = (Wkv[:D].astype(np.float64) @ b64 + bkv[:D]).astype(np.float32)
    bv_f = Wv64 @ b64 + bkv[D:]          # folded into bo on host (f64)

    def pair_rows(wT, width):
        # [D or OH, width] fp8 -> [rows/2, 2, width] with 128-row pairs
        # interleaved for the DoubleRow contraction layout
        n = wT.shape[0]
        return np.ascontiguousarray(
            wT.reshape(n // 256, 2, P, width).transpose(0, 2, 1, 3)
            .reshape(n // 2, 2, width))

    in_maps = []
    for core in range(8):
        b, half = divmod(core, 2)
        sl = slice(half * OH, (half + 1) * OH)
        pad01 = (np.arange(S) < lens[b]).astype(np.float32)
        in_maps.append({
            "x": (x[b] * 0.5).astype(bf16),
            "wqt": pair_rows((Wq64[sl].T * 16.0).astype(fp8), OH),
            "wkt": pair_rows((Wk64[sl].T * 16.0).astype(fp8), OH),
            "wvt": pair_rows((Wv64[sl].T * 16.0).astype(fp8), OH),
            "wot": pair_rows((Wo[:, sl].T * 16.0).astype(fp8), D),
            "bq": np.ascontiguousarray((16.0 * bq_f[sl]).reshape(NOT, P).T),
            "bk": np.ascontiguousarray((16.0 * bk_f[sl]).reshape(NOT, P).T),
            "pad01": np.ascontiguousarray(pad01.reshape(NST, P).T),
        })
    bo_eff = (np.asarray(bo, np.float64)
              + np.asarray(Wo, np.float64) @ bv_f).astype(np.float32)
    return in_maps, bo_eff


def kernel(**inputs):
    from concourse.bass_utils import run_bass_kernel_spmd

    trace = bool(os.environ.get("KERNEL_TRACE"))
    if trace:
        try:
            import antenv.axon_hooks  # noqa: F401  (profiling shim present?)
        except ImportError:
            trace = False
    nc = _get_nc()
    in_maps, bo_eff = make_in_maps(**inputs)
    res = run_bass_kernel_spmd(
        nc, in_maps, core_ids=list(range(8)), trace=trace)
    _CACHE["last_results"] = res
    y = np.empty((B, S, D), dtype=np.float32)
    for b in range(B):
        y[b] = res.results[2 * b]["out"] + res.results[2 * b + 1]["out"] + bo_eff
    return y


# revision 26
# speedup vs baseline: 1.2132x; 1.0192x over previous
"""Trainium2 Bass kernel: causal multi-head self-attention block (pre-LN).

Full module computed on 8 NeuronCores:
    xn = LayerNorm(x); q = xn@Wq.T+bq; k,v = xn@Wkv.T+bkv
    out = softmax(mask(q k^T / sqrt(dh))) v @ Wo.T + bo + x
Sharding: core = batch_index * 2 + head_half.  Each core handles one batch
element and 8 of the 16 heads (column-parallel QKV, row-parallel Wo), emits a
partial [S, D] output including half the residual; host sums core pairs and
adds bo (with the V-bias contribution folded in on host: exact algebra).

v4: single ACT table set (rstd = exp(-0.5 ln(var+eps))); one wide ACTIVATE per
[P,2,QS] score tile, windowed past the fully-masked half of diagonal pairs;
causal mask as a static-fp8-tile multiply (r2 pairs draw from a zero-
initialized pt pool so the skipped window stays 0); x resident in SBUF as
bf16; software-pipelined slots (QK of slot i+1 -> paced helper work -> AV of
slot i) so the PE always has independent queued work while Scalar streams exp.
"""

import os
import sys

import numpy as np

sys.path.insert(0, "/opt/trn_rl_repo")

B, S, D, H = 4, 2048, 1024, 16
DH = D // H            # 64
HL = H // 2            # heads per core: 8
OH = HL * DH           # per-core head features: 512
EPS = 1e-5
P = 128                # SBUF partitions
NST = S // P           # 16 s-tiles
NFT = D // P           # 8 feature tiles
NFP = NFT // 2         # 4 paired feature tiles (DoubleRow)
NOT = OH // P          # 4 o-tiles (per-core head features)
NOP = NOT // 2         # 2 paired o-tiles
NVP = NST // 2         # 8 paired v key-tiles
QS = 512               # query super-tile (matmul moving free dim)
NQS = S // QS          # 4
KT_PER_QS = QS // P    # 4 k-tiles per q-super
VW = 80                # per-head stride in the padded V tiles (16B-aligned)
ESC = 0.125 / 256.0    # exp scale: 1/sqrt(DH) plus 1/(16*16) weight descale

_CACHE = {}


def _build_nc():
    import concourse.bass as bass
    import concourse.bacc as bacc
    import concourse.tile as tile
    from concourse import mybir

    f32 = mybir.dt.float32
    bf16 = mybir.dt.bfloat16
    fp8 = mybir.dt.float8e4
    Alu = mybir.AluOpType
    Act = mybir.ActivationFunctionType
    DR = mybir.MatmulPerfMode.DoubleRow

    nc = bacc.Bacc("TRN2", target_bir_lowering=False, debug=False, num_devices=8)

    # ---- DRAM I/O (per-core shard shapes; weights pre-paired + fp8, x16) ----
    x_d = nc.dram_tensor("x", [S, D], bf16, kind="ExternalInput").ap()  # x/2
    wq_d = nc.dram_tensor("wqt", [NFP * P, 2, OH], fp8, kind="ExternalInput").ap()
    wk_d = nc.dram_tensor("wkt", [NFP * P, 2, OH], fp8, kind="ExternalInput").ap()
    wv_d = nc.dram_tensor("wvt", [NFP * P, 2, OH], fp8, kind="ExternalInput").ap()
    wo_d = nc.dram_tensor("wot", [NOP * P, 2, D], fp8, kind="ExternalInput").ap()
    bq_d = nc.dram_tensor("bq", [P, NOT], f32, kind="ExternalInput").ap()   # x16
    bk_d = nc.dram_tensor("bk", [P, NOT], f32, kind="ExternalInput").ap()   # x16
    pad_d = nc.dram_tensor("pad01", [P, NST], f32, kind="ExternalInput").ap()
    out_d = nc.dram_tensor("out", [S, D], f32, kind="ExternalOutput").ap()

    def headview(ap_2d, stride, width):
        # [P, *] AP -> [P, HL, width] view with per-head stride
        return bass.AP(tensor=ap_2d.tensor, offset=ap_2d.offset,
                       ap=[ap_2d.ap[0], [stride, HL], [1, width]])

    with tile.TileContext(nc) as tc:
        with (
            tc.tile_pool(name="res", bufs=1) as res,       # resident tensors
            tc.tile_pool(name="small", bufs=4) as small,
        ):
            # ---------- constants ----------
            pad_sb = res.tile([P, NST], f32, tag="pad_sb")
            zero_sb = res.tile([P, 1], f32, tag="zero_sb")
            nc.vector.memset(zero_sb, 0.0)
            ident_b = res.tile([P, P], bf16, tag="ident_b")
            nc.gpsimd.memset(ident_b, 0.0)
            nc.gpsimd.affine_select(
                out=ident_b, in_=ident_b, compare_op=Alu.not_equal, fill=1.0,
                base=0, pattern=[[-1, P]], channel_multiplier=1)
            bq_sb = res.tile([P, NOT], f32, tag="bq_sb")
            bk_sb = res.tile([P, NOT], f32, tag="bk_sb")
            rstd_all = res.tile([P, NST], f32, tag="rstd_all")
            mb_all = res.tile([P, NST], f32, tag="mb_all")
            mv_all = res.tile([P, 2, NST], f32, tag="mv_all")
            qk1 = res.tile([P, NST], f32, tag="qk1")   # quake-rsqrt scratch
            qk2 = res.tile([P, NST], f32, tag="qk2")

            # ---------- resident big tensors ----------
            x_res = [res.tile([P, D], bf16, tag=f"xr{st}", name=f"xr{st}")
                     for st in range(NST)]
            xnT = [res.tile([P, 2, S], fp8, tag=f"xnT{j}", name=f"xnT{j}")
                   for j in range(NFP)]
            qT = [res.tile([P, S], bf16, tag=f"qT{t}", name=f"qT{t}")
                  for t in range(NOT)]
            kT = [res.tile([P, S], bf16, tag=f"kT{t}", name=f"kT{t}")
                  for t in range(NOT)]
            # V pairs augmented with a pad column per head at h*VW+64;
            # VW=80 keeps the DoubleRow ldweights 16B-aligned
            vp = [res.tile([P, 2, HL * VW], fp8, tag=f"vp{i}",
                           name=f"vp{i}") for i in range(NVP)]
            oT = [res.tile([P, 2, S], fp8, tag=f"oT{t}", name=f"oT{t}")
                  for t in range(NOP)]
            wqT = [res.tile([P, 2, OH], fp8, tag=f"wqT{j}", name=f"wqT{j}")
                   for j in range(NFP)]
            wkT = [res.tile([P, 2, OH], fp8, tag=f"wkT{j}", name=f"wkT{j}")
                   for j in range(NFP)]
            wvT = [res.tile([P, 2, OH], fp8, tag=f"wvT{j}", name=f"wvT{j}")
                   for j in range(NFP)]
            woT = [res.tile([P, 2, D], fp8, tag=f"woT{t}", name=f"woT{t}")
                   for t in range(NOP)]

            with (
                tc.tile_pool(name="pj_psum", bufs=2, space="PSUM") as pp,
                tc.tile_pool(name="s_psum", bufs=2, space="PSUM") as sp,
                tc.tile_pool(name="o_psum", bufs=2, space="PSUM") as op,
                tc.tile_pool(name="pt", bufs=6) as ptp,
                tc.tile_pool(name="pt2", bufs=2) as pt2p,
                tc.tile_pool(name="nrm", bufs=2) as nrm,
                tc.tile_pool(name="tmp", bufs=3) as tmp,
                tc.tile_pool(name="ye", bufs=3) as yp,
            ):
                # ---------- prologue DMAs ----------
                nc.sync.dma_start(out=pad_sb, in_=pad_d)
                nc.sync.dma_start(out=bq_sb, in_=bq_d)
                nc.sync.dma_start(out=bk_sb, in_=bk_d)
                for st in range(8):
                    nc.sync.dma_start(out=x_res[st],
                                      in_=x_d[st * P:(st + 1) * P, :])
                for j in range(NFP):
                    nc.sync.dma_start(out=wqT[j], in_=wq_d[j * P:(j + 1) * P, :, :])
                    nc.sync.dma_start(out=wkT[j], in_=wk_d[j * P:(j + 1) * P, :, :])
                    nc.sync.dma_start(out=wvT[j], in_=wv_d[j * P:(j + 1) * P, :, :])
                for t in range(NOP):
                    nc.sync.dma_start(out=woT[t], in_=wo_d[t * P:(t + 1) * P, :, :])
                for st in range(8, NST):
                    nc.sync.dma_start(out=x_res[st],
                                      in_=x_d[st * P:(st + 1) * P, :])

                # V tiles: 1.0 everywhere, then the per-head pad column gets
                # the 0/1 pad value for its two key tiles.  Only the tiles
                # chunk-0 prep touches are initialized inline; the rest pop
                # from the helper queue so the first transposes start early.
                def vp_init(i):
                    nc.gpsimd.memset(vp[i], 1.0)
                    for st in (2 * i, 2 * i + 1):
                        vsl = vp[st // 2][:, st % 2, :]
                        ones = bass.AP(
                            tensor=vsl.tensor, offset=vsl.offset + DH,
                            ap=[vsl.ap[0], [VW, HL], [1, 1]])
                        nc.gpsimd.tensor_scalar_mul(
                            out=ones, in0=ones,
                            scalar1=pad_sb[:, st:st + 1])
                for i in range(2):
                    vp_init(i)

                # pre-zero the diagonal-r2 pt buffers: their [0, 2P) window
                # is never written, so AV reads zeros there forever
                for _ in range(2):
                    z = pt2p.tile([P, 2, QS], fp8, tag="pt2")
                    nc.vector.memset(z, 0.0)

                xns = {}
                i32 = mybir.dt.int32

                def ln_stats(st):
                    stats = small.tile([P, 2, 6], f32, tag="stats")
                    for sg in range(2):
                        nc.vector.bn_stats(
                            out=stats[:, sg, :],
                            in_=x_res[st][:, sg * 512:(sg + 1) * 512])
                    nc.vector.bn_aggr(out=mv_all[:, :, st], in_=stats)

                def ln_rstd(lo, hi):
                    # rstd = 1/sqrt(var+eps) entirely on DVE for tiles
                    # [lo,hi): Quake bitcast seed + 2 Newton iterations
                    # (~5e-6 rel err).  No ScalarE table swap traffic.
                    v = qk1[:, lo:hi]
                    y = qk2[:, lo:hi]
                    mean = mv_all[:, 0, lo:hi]
                    var = mv_all[:, 1, lo:hi]
                    nc.vector.tensor_scalar(
                        out=v, in0=var, scalar1=0.0, scalar2=EPS / 4.0,
                        op0=Alu.max, op1=Alu.add)
                    nc.vector.tensor_scalar(
                        out=y.bitcast(i32), in0=v.bitcast(i32),
                        scalar1=1, scalar2=None,
                        op0=Alu.logical_shift_right)
                    nc.vector.tensor_scalar(
                        out=y.bitcast(i32), in0=y.bitcast(i32),
                        scalar1=-1, scalar2=0x5F3759DF,
                        op0=Alu.mult, op1=Alu.add)
                    r = rstd_all[:, lo:hi]
                    for _ in range(2):
                        t = mb_all[:, lo:hi]   # scratch before mb written
                        nc.vector.tensor_mul(out=t, in0=y, in1=y)
                        nc.vector.tensor_mul(out=t, in0=t, in1=v)
                        nc.vector.tensor_scalar(
                            out=t, in0=t, scalar1=-0.5, scalar2=1.5,
                            op0=Alu.mult, op1=Alu.add)
                        nc.vector.tensor_mul(out=y, in0=y, in1=t)
                    nc.vector.tensor_copy(out=r, in_=y)
                    # mb = -mean*rstd
                    nc.vector.tensor_scalar(
                        out=mb_all[:, lo:hi], in0=mean, scalar1=-1.0,
                        scalar2=None, op0=Alu.mult)
                    nc.vector.tensor_mul(out=mb_all[:, lo:hi],
                                         in0=mb_all[:, lo:hi], in1=r)

                def ln_compute(st):
                    xn = tmp.tile([P, D], bf16, tag="xn", bufs=9)
                    nc.vector.tensor_scalar(
                        out=xn, in0=x_res[st],
                        scalar1=rstd_all[:, st:st + 1],
                        scalar2=mb_all[:, st:st + 1],
                        op0=Alu.mult, op1=Alu.add)
                    xns[st] = xn

                def ln_transpose(st):
                    # 4 transposes share one PSUM tile; 2 paired copies out
                    xn = xns.pop(st)
                    for g in range(2):
                        ps = pp.tile([P, 4, P], bf16, tag="pj")
                        for m in range(4):
                            j = 4 * g + m
                            nc.tensor.transpose(
                                ps[:, m, :], xn[:, j * P:(j + 1) * P],
                                ident_b)
                        for h in range(2):
                            nc.vector.tensor_copy(
                                out=xnT[2 * g + h][:, :,
                                                   st * P:(st + 1) * P],
                                in_=ps[:, 2 * h:2 * h + 2, :])

                def qk_proj(c, wT, dst, bias, t):
                    ps = pp.tile([P, QS], f32, tag="pj")
                    for j in range(NFP):
                        nc.tensor.matmul(
                            ps,
                            lhsT=wT[j][:, :, t * P:(t + 1) * P],
                            rhs=xnT[j][:, :, c * QS:(c + 1) * QS],
                            start=(j == 0), stop=(j == NFP - 1),
                            perf_mode=DR)
                    nc.vector.tensor_scalar_add(
                        out=dst[t][:, c * QS:(c + 1) * QS],
                        in0=ps, scalar1=bias[:, t:t + 1])

                def v_proj(st):
                    ps = pp.tile([P, OH], f32, tag="pj")
                    for j in range(NFP):
                        nc.tensor.matmul(
                            ps,
                            lhsT=xnT[j][:, :, st * P:(st + 1) * P],
                            rhs=wvT[j],
                            start=(j == 0), stop=(j == NFP - 1),
                            perf_mode=DR)
                    vsl = vp[st // 2][:, st % 2, :]
                    # v = raw*0.0625*pad (vb folded into bo on host)
                    nc.vector.tensor_scalar(
                        out=headview(vsl, VW, DH),
                        in0=headview(ps[:, :], DH, DH),
                        scalar1=0.0625, scalar2=pad_sb[:, st:st + 1],
                        op0=Alu.mult, op1=Alu.mult)

                def outproj(st, mc):
                    ps = pp.tile([P, QS], f32, tag="pj")
                    for ot in range(NOP):
                        nc.tensor.matmul(
                            ps,
                            lhsT=oT[ot][:, :, st * P:(st + 1) * P],
                            rhs=woT[ot][:, :, mc * QS:(mc + 1) * QS],
                            start=(ot == 0), stop=(ot == NOP - 1),
                            perf_mode=DR)
                    y_sb = yp.tile([P, QS], f32, tag="y_sb")
                    nc.vector.scalar_tensor_tensor(
                        out=y_sb, in0=ps, scalar=0.0625,
                        in1=x_res[st][:, mc * QS:(mc + 1) * QS],
                        op0=Alu.mult, op1=Alu.add)
                    nc.sync.dma_start(
                        out=out_d[st * P:(st + 1) * P,
                                  mc * QS:(mc + 1) * QS],
                        in_=y_sb)

                # ---------- helper work queue (PE filler) ----------
                # entries: (deadline_chunk, pe_cost_ns, fn)
                queue = []
                popped = [0.0]
                budget = [0.0]

                def push_prep(c):
                    c0 = c * KT_PER_QS
                    for st in range(c0, c0 + KT_PER_QS):
                        queue.append((c, 2200.0, lambda st=st: ln_transpose(st)))
                        queue.append((c, 964.0, lambda st=st: v_proj(st)))
                    for (wT, dst, bias) in ((wqT, qT, bq_sb),
                                            (wkT, kT, bk_sb)):
                        for t in range(NOT):
                            queue.append(
                                (c, 964.0,
                                 lambda wT=wT, dst=dst, bias=bias, t=t, c=c:
                                 qk_proj(c, wT, dst, bias, t)))

                def push_outproj(c):
                    for st in range(c * KT_PER_QS, (c + 1) * KT_PER_QS):
                        for mc in range(2):
                            queue.append(
                                (99, 600.0,
                                 lambda st=st, mc=mc: outproj(st, mc)))

                def pop_paced():
                    while queue and popped[0] < budget[0]:
                        _, cost, fn = queue.pop(0)
                        fn()
                        popped[0] += cost

                def drain_deadline(c):
                    keep = []
                    for ent in queue:
                        if ent[0] <= c:
                            ent[2]()
                            popped[0] += ent[1]
                        else:
                            keep.append(ent)
                    queue[:] = keep

                # ---------- attention slot machinery ----------
                def emit_qk(qs, hp, kt0):
                    # 4 QK matmuls; h0 rows 0-63 and h1 rows 64-127 run
                    # row-tile concurrent into separate PSUM tiles
                    tiles = []
                    for hh in range(2):
                        h = 2 * hp + hh
                        hq = qT[h // 2][(h % 2) * DH:(h % 2) * DH + DH, :]
                        hk = kT[h // 2][(h % 2) * DH:(h % 2) * DH + DH, :]
                        s_t = sp.tile([P, 2, QS], f32, tag="s_ps")
                        for i in range(2):
                            kt = kt0 + i
                            nc.tensor.matmul(
                                s_t[:, i, :],
                                lhsT=hk[:, kt * P:(kt + 1) * P],
                                rhs=hq[:, qs * QS:(qs + 1) * QS],
                                start=True, stop=True, skip_group_check=True)
                        tiles.append(s_t)
                    return tiles

                def emit_exp(qs, kt0, s_tiles):
                    diag = kt0 >= qs * KT_PER_QS
                    r2 = diag and (kt0 - qs * KT_PER_QS) == 2
                    ws = 2 * P if r2 else 0
                    pts = []
                    for s_t in s_tiles:
                        pool = pt2p if r2 else ptp
                        pt = pool.tile([P, 2, QS], fp8,
                                       tag="pt2" if r2 else "pt")
                        if r2:
                            nc.vector.memset(pt[:, :, :ws], 0.0)
                        nc.scalar.activation(
                            out=pt[:, :, ws:], in_=s_t[:, :, ws:],
                            func=Act.Exp, bias=zero_sb, scale=ESC)
                        if diag:
                            nc.gpsimd.affine_select(
                                out=pt[:, :, ws:ws + 2 * P],
                                in_=pt[:, :, ws:ws + 2 * P],
                                compare_op=Alu.is_ge, fill=0.0, base=0,
                                pattern=[[-P, 2], [1, 2 * P]],
                                channel_multiplier=-1)
                        pts.append(pt)
                    return pts

                def emit_av(qs, hp, kt0, pts, o_tiles, nkt):
                    for hh in range(2):
                        h = 2 * hp + hh
                        nc.tensor.matmul(
                            o_tiles[hh],
                            lhsT=vp[kt0 // 2][:, :,
                                              h * VW:h * VW + DH + 2],
                            rhs=pts[hh],
                            start=(kt0 == 0), stop=(kt0 + 2 == nkt),
                            perf_mode=DR, skip_group_check=True)

                def normalize(qs, hp, o_tiles):
                    for hh in range(2):
                        h = 2 * hp + hh
                        ops = o_tiles[hh]
                        # reciprocal_approx_fast misreads PSUM inputs; stage
                        # den through SBUF on the Scalar engine (Copy is in
                        # every ACT table set, so no table swap)
                        den = nrm.tile([1, QS], f32, tag="den")
                        nc.scalar.copy(out=den, in_=ops[DH:DH + 1, :])
                        dbc = nrm.tile([DH, QS], f32, tag="dbc")
                        nc.vector.reciprocal_approx_fast(
                            out=dbc[0:1, :], in_=den)
                        nc.gpsimd.partition_broadcast(dbc, dbc[0:1, :])
                        nc.vector.tensor_mul(
                            out=oT[h // 4][(h % 2) * DH:(h % 2) * DH + DH,
                                           (h // 2) % 2,
                                           qs * QS:(qs + 1) * QS],
                            in0=ops[0:DH, :], in1=dbc)

                # ---------- prologue compute ----------
                # minimal critical path to the first transposes
                ln_stats(0)
                ln_stats(1)
                ln_rstd(0, 2)
                ln_compute(0)
                ln_compute(1)
                ln_transpose(0)
                v_proj(0)
                ln_stats(2)
                ln_stats(3)
                ln_rstd(2, 4)
                ln_compute(2)
                ln_compute(3)
                ln_transpose(1)
                v_proj(1)
                ln_stats(4)
                ln_stats(5)
                ln_transpose(2)
                v_proj(2)
                ln_stats(6)
                ln_stats(7)
                ln_rstd(4, 8)
                ln_transpose(3)
                v_proj(3)
                for (wT, dst, bias) in ((wqT, qT, bq_sb), (wkT, kT, bk_sb)):
                    for t in range(NOT):
                        qk_proj(0, wT, dst, bias, t)
                for st in range(4, 8):
                    ln_compute(st)
                for i in range(2, NVP):
                    queue.append((1, 0.0, lambda i=i: vp_init(i)))
                push_prep(1)
                # stats + rstd for tiles 8-15 pop early in chunk 0 (DVE-only)
                for st0 in range(8, NST, 2):
                    queue.append((1, 0.0, lambda st0=st0: (
                        ln_stats(st0), ln_stats(st0 + 1))))
                queue.append((1, 0.0, lambda: ln_rstd(8, NST)))

                # ---------- pipelined attention ----------
                slots = []
                for c in range(NQS):
                    nkt = (c + 1) * KT_PER_QS
                    for hp in range(HL // 2):
                        for kt0 in range(0, nkt, 2):
                            slots.append((c, hp, kt0, nkt))

                # AV trails its QK/exp by TWO slots: the h-pair-boundary
                # o_psum WAR then has a full slot of slack to cover the
                # normalize chain latency (copy->recip->bcast->mul)
                pendings = []

                def flush_av():
                    pq, php, pkt0, ppts, po, pnkt, plast = pendings.pop(0)
                    emit_av(pq, php, pkt0, ppts, po, pnkt)
                    if plast:
                        normalize(pq, php, po)
                        if php == HL // 2 - 1:
                            push_outproj(pq)

                o_cur = None
                cur_chunk = -1
                for (c, hp, kt0, nkt) in slots:
                    if c != cur_chunk:
                        cur_chunk = c
                        if c > 0:
                            drain_deadline(c)
                        if c + 2 < NQS:
                            for st in range((c + 2) * KT_PER_QS,
                                            (c + 3) * KT_PER_QS):
                                queue.append(
                                    (c + 1, 0.0,
                                     lambda st=st: ln_compute(st)))
                            push_prep(c + 2)
                    if kt0 == 0:
                        o_cur = [op.tile([DH + 2, QS], f32, tag="o_ps",
                                         name="o_ps0"),
                                 op.tile([DH + 2, QS], f32, tag="o_ps",
                                         name="o_ps1")]
                    s_tiles = emit_qk(c, hp, kt0)
                    pts = emit_exp(c, kt0, s_tiles)
                    # pace helper work between QK and the pending AV;
                    # eager: prep should finish mid-chunk, not at the edge
                    budget[0] += 2200.0
                    pop_paced()
                    if len(pendings) >= 2:
                        flush_av()
                    pendings.append((c, hp, kt0, pts, o_cur, nkt,
                                     kt0 + 2 == nkt))
                while pendings:
                    flush_av()

                # epilogue: drain all remaining helper work
                for (_, _, fn) in queue:
                    fn()

    nc.compile()
    return nc


def _get_nc():
    if "nc" not in _CACHE:
        _CACHE["nc"] = _build_nc()
    return _CACHE["nc"]


def make_in_maps(x, key_val_lengths, Wq, bq, Wkv, bkv, Wo, bo, ln_g, ln_b):
    import ml_dtypes
    fp8 = ml_dtypes.float8_e4m3
    bf16 = ml_dtypes.bfloat16

    x = np.ascontiguousarray(np.asarray(x, dtype=np.float32))
    lens = np.asarray(key_val_lengths).astype(np.int64)
    Wq = np.asarray(Wq, dtype=np.float32)
    Wkv = np.asarray(Wkv, dtype=np.float32)
    Wo = np.asarray(Wo, dtype=np.float32)
    bq = np.asarray(bq, dtype=np.float32)
    bkv = np.asarray(bkv, dtype=np.float32)
    ln_g = np.asarray(ln_g, dtype=np.float32)
    ln_b = np.asarray(ln_b, dtype=np.float32)

    # fold LayerNorm gain into the projection weights and shift into the
    # biases (exact algebra): q = ((x-mu)rstd*g + b) @ Wq.T + bq
    #                           = xhat @ (Wq*g).T + (Wq@b + bq)
    g64 = ln_g.astype(np.float64)
    b64 = ln_b.astype(np.float64)
    Wq64 = Wq.astype(np.float64) * g64
    Wk64 = Wkv[:D].astype(np.float64) * g64
    Wv64 = Wkv[D:].astype(np.float64) * g64
    bq_f = (Wq.astype(np.float64) @ b64 + bq).astype(np.float32)
    bk_f = (Wkv[:D].astype(np.float64) @ b64 + bkv[:D]).astype(np.float32)
    bv_f = Wv64 @ b64 + bkv[D:]          # folded into bo on host (f64)

    def pair_rows(wT, width):
        # [D or OH, width] fp8 -> [rows/2, 2, width] with 128-row pairs
        # interleaved for the DoubleRow contraction layout
        n = wT.shape[0]
        return np.ascontiguousarray(
            wT.reshape(n // 256, 2, P, width).transpose(0, 2, 1, 3)
            .reshape(n // 2, 2, width))

    in_maps = []
    for core in range(8):
        b, half = divmod(core, 2)
        sl = slice(half * OH, (half + 1) * OH)
        pad01 = (np.arange(S) < lens[b]).astype(np.float32)
        in_maps.append({
            "x": (x[b] * 0.5).astype(bf16),
            "wqt": pair_rows((Wq64[sl].T * 16.0).astype(fp8), OH),
            "wkt": pair_rows((Wk64[sl].T * 16.0).astype(fp8), OH),
            "wvt": pair_rows((Wv64[sl].T * 16.0).astype(fp8), OH),
            "wot": pair_rows((Wo[:, sl].T * 16.0).astype(fp8), D),
            "bq": np.ascontiguousarray((16.0 * bq_f[sl]).reshape(NOT, P).T),
            "bk": np.ascontiguousarray((16.0 * bk_f[sl]).reshape(NOT, P).T),
            "pad01": np.ascontiguousarray(pad01.reshape(NST, P).T),
        })
    bo_eff = (np.asarray(bo, np.float64)
              + np.asarray(Wo, np.float64) @ bv_f).astype(np.float32)
    return in_maps, bo_eff


def kernel(**inputs):
    from concourse.bass_utils import run_bass_kernel_spmd

    trace = bool(os.environ.get("KERNEL_TRACE"))
    if trace:
        try:
            import antenv.axon_hooks  # noqa: F401  (profiling shim present?)
        except ImportError:
            trace = False
    nc = _get_nc()
    in_maps, bo_eff = make_in_maps(**inputs)
    res = run_bass_kernel_spmd(
        nc, in_maps, core_ids=list(range(8)), trace=trace)
    _CACHE["last_results"] = res
    y = np.empty((B, S, D), dtype=np.float32)
    for b in range(B):
        y[b] = res.results[2 * b]["out"] + res.results[2 * b + 1]["out"] + bo_eff
    return y
